# revision 4
# baseline (speedup 1.0000x reference)
"""Trainium2 Bass kernel for nn_CMambaSlim.

Strategy (8 NeuronCores):
  - Data-parallel trunk: each core runs the CMamba trunk (patch embed, 4
    mamba blocks, final RMSNorm) on B/8 = 4 batch samples, fp32/f32r.
  - AllGather of the flattened features (1 MB bf16) across the 8 cores.
  - Tensor-parallel output layer: core r streams rows [r*768, (r+1)*768) of
    out_W in bf16 (pre-transposed on host to [16000, 768]) and computes
    y[:, r*768:(r+1)*768]. out_b is added on the host during unsharding.
  The host concatenates the 8 output slices and adds the bias.

Schedule notes (CoreSim cost model):
  - All DMAs serialize on the DMA-engine device, and a DMA holds its issuing
    engine's sequencer for the whole transfer. SP's queue therefore carries
    ONLY the wpack loads + the 32 streamed weight chunks, so weight
    prefetch continues through the trunk and the AllGather. ccin/flatT/y
    DMAs issue from the Activation engine.
  - The weight ring (18 bufs x 4 k-slices, bf16) covers the trunk+gather
    window, leaving the output matmul PE-bound rather than DMA-bound.
  - ACT stays on the silu table the whole trunk (Silu/Copy/Square); rstd is
    computed on DVE as (ssum + D*eps)^-0.5, avoiding Sqrt table thrash.
"""

import math
import os
import sys

import numpy as np

for _p in ("/opt/trn_rl_repo", "/root/.axon_site/_ro/trn_rl_repo"):
    if os.path.isdir(_p) and _p not in sys.path:
        sys.path.insert(0, _p)
        break

import concourse.bass as bass
import concourse.tile as tile
from concourse import mybir
from concourse.bass_utils import run_bass_kernel_spmd

# Model dims (hardcoded per problem spec)
B, C, L = 32, 64, 512
P, S = 16, 4
NP = 125
D = 128
INNER = 256
K5 = 5
NL = 4
F = 96
EPS = 1e-5

NCORES = 8
BLOC = B // NCORES            # 4 samples per core
OSL = (C * F) // NCORES       # 768 output cols per core
TOK = BLOC * 128              # padded token span (125 valid + 3 pad per sample)
TOKW = TOK + 4                # + 2 guard cols each side
LPAD = 520                    # x padded along L so the +8-shifted copy stays in bounds
NF = NP * D                   # 16000 contraction size
WKC = 4                       # k-chunks per weight-stream DMA
NQ = (NP + WKC - 1) // WKC    # 32 chunks (31 full + 1 partial)
WRING = 18                    # weight ring depth (chunks prefetchable)

f32 = mybir.dt.float32
f32r = mybir.dt.float32r
bf16 = mybir.dt.bfloat16
AF = mybir.ActivationFunctionType
OP = mybir.AluOpType

_PROG = None

SQRTD = math.sqrt(float(D))

# wpack layout (fp32, loaded as 5 DMAs: embed part + one per layer)
WOFF_PE8 = BLOC * 512                      # 2048 cols of x
WOFF_POS = WOFF_PE8 + 8 * 128              # 1024 cols of patch-embed W
WOFF_CON = WOFF_POS + 129                  # 129 cols: sincos+pe_b (+normf col)
WOFF_L0 = WOFF_CON + 129                   # 129 cols: sqrtD block + ones col
LCOLS = K5 * INNER + INNER + 2 * D + 8     # 1800 cols per layer
WCOLS = WOFF_L0 + NL * LCOLS               # 10530


def build_program():
    nc = bass.Bass(num_devices=NCORES)

    wpack = nc.declare_dram_parameter("wpack", [128, WCOLS], f32, isOutput=False)
    wt = nc.declare_dram_parameter("wt", [NF, OSL], bf16, isOutput=False)
    y = nc.declare_dram_parameter("y", [B, OSL], f32, isOutput=True)

    wtT = wt[:].tensor

    with tile.TileContext(nc) as tc:
        with (
            tc.tile_pool(name="const", bufs=1) as const,
            tc.tile_pool(name="work", bufs=1) as work,
            tc.tile_pool(name="wring", bufs=WRING) as wring,
            tc.tile_pool(name="ps", bufs=1, space="PSUM") as ps,
            tc.tile_pool(name="dram", bufs=1, space="DRAM") as dram,
        ):
            # ---------------- constant loads (embed part, then per layer) ----
            wp = const.tile([128, WCOLS], f32r)
            nc.sync.dma_start(out=wp[:, 0:WOFF_L0],
                              in_=wpack[:, 0:WOFF_L0].bitcast(f32r))
            for l in range(NL):
                c0 = WOFF_L0 + l * LCOLS
                nc.sync.dma_start(out=wp[:, c0:c0 + LCOLS],
                                  in_=wpack[:, c0:c0 + LCOLS].bitcast(f32r))

            xO4 = wp[:, 0:WOFF_PE8].rearrange(
                "p (b k s) -> p b k s", b=BLOC, s=4)          # [128, 4, 128, 4]
            pe8sb = wp[:, WOFF_PE8:WOFF_POS].rearrange("p (j d) -> p j d", j=8)
            miscsb = wp[:, WOFF_POS:WOFF_CON].bitcast(f32)     # posb + normf col
            sqrtDrow = wp[0:1, WOFF_CON:WOFF_CON + 128]        # value sqrt(D)
            onesD = wp[:, WOFF_CON + 128:WOFF_CON + 129]       # value 1.0

            def lview(l):
                b0 = WOFF_L0 + l * LCOLS
                w5 = wp[:, b0:b0 + K5 * INNER].rearrange(
                    "p (k i) -> p k i", k=K5)
                bw = wp[:, b0 + K5 * INNER:b0 + K5 * INNER + INNER]
                ow = wp[:, b0 + K5 * INNER + INNER:
                        b0 + K5 * INNER + INNER + 2 * D].rearrange(
                    "p (c d) -> p c d", c=2)
                scal = wp[:, b0 + LCOLS - 8:b0 + LCOLS].bitcast(f32).rearrange(
                    "p (s c) -> p s c", s=4)
                return w5, bw, ow, scal

            # mask01: 1 everywhere, 0 at each sample's k=0 column (scan reset)
            mask01 = const.tile([128, TOK], f32)
            nc.vector.memset(mask01[:], 1.0)
            for bq in range(BLOC):
                nc.vector.memset(mask01[:, bq * 128:bq * 128 + 1], 0.0)

            # residual stream h: [d, 2 guard + (b,k) + 2 guard], fp32
            h = const.tile([128, TOKW], f32)
            nc.vector.memset(h[:], 0.0)
            h_tok = h[:, 2:2 + TOK]
            h_bk = h_tok.rearrange("p (b k) -> p b k", b=BLOC)

            # normalized-input tile, guards zeroed once (h guards stay zero)
            hn = const.tile([128, TOKW], f32)
            nc.vector.memset(hn[:, 0:2], 0.0)
            nc.vector.memset(hn[:, 2 + TOK:], 0.0)
            hnr = hn[:].bitcast(f32r)

            # ---------------- patch embedding ----------------
            ph = ps.tile([128, BLOC, 126], f32, tag="prstd")
            for j in range(8):
                jq, jr = j // 4, j % 4
                rhs = xO4[:, :, jq:jq + 126, jr]
                nc.tensor.matmul(
                    out=ph[:], lhsT=pe8sb[:, j, :],
                    rhs=rhs, start=(j == 0), stop=(j == 7))
            posb = miscsb[:, 0:NP].unsqueeze(1).broadcast_to([128, BLOC, NP])
            nc.vector.tensor_tensor(
                out=h_bk[:, :, 0:NP], in0=ph[:, :, 0:NP], in1=posb, op=OP.add)

            # ---------------- mamba layers ----------------
            for l in range(NL):
                w5sb, bwsb, owsb, scalsb = lview(l)
                # rmsnorm stats: ssum = sum_d h^2 (PE ones-reduce on squared h)
                sq = work.tile([128, TOK], f32, tag="sq", name="t_sq")
                nc.scalar.activation(out=sq[:], in_=h_tok, func=AF.Square)
                pssum = ps.tile([1, TOK], f32, tag="py", bufs=2, name="t_pssum")
                nc.tensor.matmul(out=pssum[:], lhsT=onesD,
                                 rhs=sq[:].bitcast(f32r), start=True, stop=True)
                # rstd' = (ssum + D*eps)^-0.5  (the 1/sqrt(D) factor is folded
                # into the sqrt(D)-valued broadcast matmul below)
                rstd = work.tile([1, TOK], f32, tag="rstd", name="t_rstd")
                nc.vector.tensor_scalar(
                    out=rstd[:], in0=pssum[:], scalar1=float(D) * EPS,
                    scalar2=-0.5, op0=OP.add, op1=OP.pow)
                prstd = ps.tile([128, TOK], f32, tag="prstd", name="t_prstd")
                nc.tensor.matmul(out=prstd[:], lhsT=sqrtDrow,
                                 rhs=rstd[:].bitcast(f32r), start=True, stop=True)
                # hn = h * rstd  (norm_w folded into w5/bw on host)
                nc.vector.tensor_tensor(out=hn[:, 2:2 + TOK], in0=h_tok,
                                        in1=prstd[:], op=OP.mult)

                # aM = mask * sigmoid(alpha): ACT copy with per-channel scale
                aM = []
                for ic in range(2):
                    t = work.tile([128, TOK], f32, tag=f"am{ic}", name=f"w{ic}_{l}")
                    nc.scalar.activation(out=t[:], in_=mask01[:], func=AF.Copy,
                                         scale=scalsb[:, 1, ic:ic + 1])
                    aM.append(t)

                # fused in-proj + depthwise conv on PE: 5 shifted matmuls
                pa = [ps.tile([128, TOK], f32, tag=f"pa{ic}", name=f"pa{ic}_{l}")
                      for ic in range(2)]
                for ic in range(2):
                    for dk in range(K5):
                        nc.tensor.matmul(
                            out=pa[ic][:],
                            lhsT=w5sb[:, dk, ic * 128:(ic + 1) * 128],
                            rhs=hnr[:, dk:dk + TOK],
                            start=(dk == 0), stop=(dk == K5 - 1))

                # b-projection (PE)
                pb = [ps.tile([128, TOK], f32, tag=f"pb{ic}", name=f"pb{ic}_{l}")
                      for ic in range(2)]
                for ic in range(2):
                    nc.tensor.matmul(
                        out=pb[ic][:],
                        lhsT=bwsb[:, ic * 128:(ic + 1) * 128],
                        rhs=hnr[:, 2:2 + TOK], start=True, stop=True)

                ab, sc = [], []
                for ic in range(2):
                    # silu(z) = z * sigmoid(z), z = conv + conv_b
                    sg = work.tile([128, TOK], f32, tag=f"sg{ic}", name=f"w{ic}_{l}")
                    nc.scalar.activation(out=sg[:], in_=pa[ic][:], func=AF.Sigmoid,
                                         bias=scalsb[:, 0, ic:ic + 1], scale=1.0)
                    u = work.tile([128, TOK], f32, tag=f"ab{ic}", name=f"w{ic}_{l}")
                    nc.vector.scalar_tensor_tensor(
                        out=u[:], in0=pa[ic][:], scalar=scalsb[:, 0, ic:ic + 1],
                        in1=sg[:], op0=OP.add, op1=OP.mult)
                    ab.append(u)
                    # scan: state = aM*state + u
                    s = work.tile([128, TOK], f32, tag=f"s{ic}", name=f"w{ic}_{l}")
                    nc.vector.tensor_tensor_scan(
                        out=s[:], data0=aM[ic][:], data1=u[:], initial=0.0,
                        op0=OP.mult, op1=OP.add)
                    sc.append(s)

                # gate: g = (gamma*beta*s + delta*u) * b
                g = []
                for ic in range(2):
                    dab = work.tile([128, TOK], f32, tag=f"dab{ic}", name=f"w{ic}_{l}")
                    nc.scalar.activation(out=dab[:], in_=ab[ic][:], func=AF.Copy,
                                         scale=scalsb[:, 3, ic:ic + 1])
                    g0 = work.tile([128, TOK], f32, tag=f"g0{ic}", name=f"w{ic}_{l}")
                    nc.vector.scalar_tensor_tensor(
                        out=g0[:], in0=sc[ic][:], scalar=scalsb[:, 2, ic:ic + 1],
                        in1=dab[:], op0=OP.mult, op1=OP.add)
                    gg = work.tile([128, TOK], f32, tag=f"g{ic}", name=f"w{ic}_{l}")
                    # pads (k=125..127) are zero because hn pads are zero, so
                    # pb pads are zero and the product zeroes them.
                    nc.vector.tensor_tensor(out=gg[:], in0=g0[:], in1=pb[ic][:],
                                            op=OP.mult)
                    g.append(gg)

                # out-projection + residual: h = 2*h + oW @ g
                py = ps.tile([128, TOK], f32, tag="py", bufs=2, name="t_py")
                for ic in range(2):
                    nc.tensor.matmul(
                        out=py[:], lhsT=owsb[:, ic, :],
                        rhs=g[ic][:].bitcast(f32r), start=(ic == 0), stop=(ic == 1))
                nc.vector.scalar_tensor_tensor(
                    out=h_tok, in0=h_tok, scalar=2.0, in1=py[:],
                    op0=OP.mult, op1=OP.add)

            # ---------------- final rmsnorm ----------------
            sqf = work.tile([128, TOK], f32, tag="sq", name="t_sq")
            nc.scalar.activation(out=sqf[:], in_=h_tok, func=AF.Square)
            pssumf = ps.tile([1, TOK], f32, tag="py", bufs=2, name="t_pssum")
            nc.tensor.matmul(out=pssumf[:], lhsT=onesD,
                             rhs=sqf[:].bitcast(f32r), start=True, stop=True)
            rstdf = work.tile([1, TOK], f32, tag="rstd", name="t_rstd")
            nc.vector.tensor_scalar(
                out=rstdf[:], in0=pssumf[:], scalar1=float(D) * EPS,
                scalar2=-0.5, op0=OP.add, op1=OP.pow)
            prstdf = ps.tile([128, TOK], f32, tag="prstd", name="t_prstd")
            nc.tensor.matmul(out=prstdf[:], lhsT=sqrtDrow,
                             rhs=rstdf[:].bitcast(f32r), start=True, stop=True)
            hf = work.tile([128, TOK], bf16, tag="hf", name="t_hf")
            nc.vector.scalar_tensor_tensor(
                out=hf[:], in0=h_tok, scalar=miscsb[:, 128:129], in1=prstdf[:],
                op0=OP.mult, op1=OP.mult)

            # ---------------- all-gather the features (bf16) ----------------
            ccin = dram.tile([128, TOK], bf16)
            nc.scalar.dma_start(out=ccin[:], in_=hf[:])
            ccout = dram.tile([NCORES, 128, TOK], bf16, addr_space="Shared")
            nc.gpsimd.collective_compute(
                "AllGather", OP.bypass,
                replica_groups=[list(range(NCORES))],
                ins=[ccin[:].opt()], outs=[ccout[:]])
            # flatT[d, b, k] (k padded to 128; pads are zero), b = r*BLOC + b4
            flatT = const.tile([128, B, 128], bf16)
            nc.scalar.dma_start(
                out=flatT[:].rearrange("p (r x) k -> p r (x k)", r=NCORES),
                in_=bass.AP(tensor=ccout[:].tensor, offset=ccout[:].offset,
                            ap=[[TOK, 128], [128 * TOK, NCORES], [1, TOK]]),
            )
            fap = flatT[:]
            fp0 = list(fap.ap[0])

            # ---------------- streamed output matmul ----------------
            # y[b, o] accumulated over the 125 (k, d) chunks. Stationary
            # operand = flatT columns (j, b) at offset k: output row j*32+b
            # holds sum_d flat[d, b, k+j] * wt_k[d, o]; rows 0..31 / j=0 are
            # the real batch rows, the rest M-padding. Moving operand = the
            # streamed bf16 W tile.
            pyb0 = ps.tile([128, 512], f32, tag="pa0", name="t_pyb0")
            pyb1 = ps.tile([128, OSL - 512], f32, tag="pa1", name="t_pyb1")
            pybs = ((pyb0, 0, 512), (pyb1, 512, OSL - 512))
            for q in range(NQ):
                kc_n = min(WKC, NP - q * WKC)
                wtl = wring.tile([128, WKC, OSL], bf16, tag="wt", name="t_wt")
                nc.sync.dma_start(
                    out=wtl[:, 0:kc_n, :],
                    in_=bass.AP(tensor=wtT, offset=q * WKC * 128 * OSL,
                                ap=[[OSL, 128], [128 * OSL, kc_n], [1, OSL]]),
                )
                for kc in range(kc_n):
                    k = q * WKC + kc
                    lhsT = bass.AP(tensor=fap.tensor, offset=fap.offset + k,
                                   ap=[fp0, [1, 4], [128, 32]])
                    for (pt, o0, on) in pybs:
                        nc.tensor.matmul(
                            out=pt[:], lhsT=lhsT,
                            rhs=wtl[:, kc, o0:o0 + on],
                            start=(k == 0), stop=(k == NP - 1),
                            skip_group_check=True)

            yout = work.tile([32, OSL], f32, tag="yout", name="t_yout")
            nc.scalar.copy(out=yout[:, 0:512], in_=pyb0[0:32, :])
            nc.scalar.copy(out=yout[:, 512:OSL], in_=pyb1[0:32, :])
            nc.scalar.dma_start(out=y[:], in_=yout[:])

    _legalize_waits(nc)
    return nc


def _legalize_waits(nc):
    """walrus on this toolchain accepts only one sync wait per non-sequencer
    instruction. Move extra waits onto standalone InstEventSemaphore
    instructions (sequencer-level waits, multi-wait legal) placed just
    before the owning instruction on the same engine."""
    n_moved = 0
    for bb in nc.main_func.blocks:
        out = []
        for inst in bb.instructions:
            si = inst.sync_info
            tn = type(inst).__name__
            if (si is not None and len(si.on_wait) > 1
                    and tn not in ("InstEventSemaphore", "InstNoOp")):
                waits = list(si.on_wait)
                for w in waits[:-1]:
                    ev = mybir.InstNoOp(
                        name=f"lw_{inst.name}_{n_moved}", ins=[], outs=[],
                        engine=inst.engine)
                    ev.sync_info = mybir.SyncInfo(on_wait=[w], on_update=[])
                    nc.register_instruction(ev)
                    out.append(ev)
                    n_moved += 1
                inst.sync_info = mybir.SyncInfo(
                    on_wait=[waits[-1]], on_update=list(si.on_update))
            out.append(inst)
        bb.instructions = out


def _sincos_pe(n, d):
    pos = np.arange(n, dtype=np.float32)[:, None]
    sin_cols, cos_cols = (d + 1) // 2, d // 2
    denom = d / 2.0
    sin_div = np.exp(
        (-math.log(10000.0) * np.arange(sin_cols, dtype=np.float32) / denom)
    ).astype(np.float32)
    cos_div = np.exp(
        (-math.log(10000.0) * np.arange(cos_cols, dtype=np.float32) / denom)
    ).astype(np.float32)
    pe = np.zeros((n, d), dtype=np.float32)
    pe[:, 0::2] = np.sin(pos * sin_div[None, :])
    pe[:, 1::2] = np.cos(pos * cos_div[None, :])
    return pe


def _to_bf16(a):
    import ml_dtypes
    return np.asarray(a, np.float32).astype(ml_dtypes.bfloat16)


def make_in_maps(x, pe_W, pe_b, norm_w, ipa_W, ipb_W, conv_W, conv_b,
                 alpha, beta, gamma, delta, op_W, normf_w, out_W, out_b):
    f = np.float32
    x = np.asarray(x, f)
    x_pad = np.zeros((B, C, LPAD), f)
    x_pad[:, :, :L] = x
    # device layout: [p2*64+c, b_loc, l] with p2=1 rows shifted by 8 along l
    xcT = x_pad.transpose(1, 0, 2)                     # [c, b, lpad]
    x_dev = np.empty((2, C, B, 512), f)
    x_dev[0] = xcT[:, :, 0:512]
    x_dev[1] = xcT[:, :, 8:520]
    x_dev = x_dev.reshape(128, B, 512)

    pw = np.asarray(pe_W, f).reshape(D, C, P)          # [d, c, p]
    t = pw.transpose(1, 2, 0)                          # [c, p, d]
    pe8 = np.ascontiguousarray(
        t.reshape(C, 2, 8, D).transpose(2, 1, 0, 3).reshape(8, 128, 128))
    pe8 = np.ascontiguousarray(pe8.transpose(1, 0, 2))  # [pp, j, d]

    posb = np.zeros((128, 129), f)
    posb[:, :NP] = _sincos_pe(NP, D).T + np.asarray(pe_b, f)[:, None]
    posb[:, 128] = np.asarray(normf_w, f)

    constb = np.full((128, 129), SQRTD, f)
    constb[:, 128] = 1.0

    nw = np.asarray(norm_w, f)                          # [NL, D]
    ipa = np.asarray(ipa_W, f)                          # [NL, INNER, D]
    cw = np.asarray(conv_W, f)[:, :, 0, :]              # [NL, INNER, K5]
    w5 = (ipa.transpose(0, 2, 1)[:, None, :, :]         # [NL, 1, D, INNER]
          * cw.transpose(0, 2, 1)[:, :, None, :]        # [NL, K5, 1, INNER]
          * nw[:, None, :, None])                       # [NL, K5, D, INNER]
    bwh = np.asarray(ipb_W, f).transpose(0, 2, 1) * nw[:, :, None]  # [NL, D, INNER]
    owh = np.asarray(op_W, f).transpose(0, 2, 1).reshape(NL, 2, 128, D)

    asig = 1.0 / (1.0 + np.exp(-np.asarray(alpha, np.float64)))
    gb = np.asarray(gamma, np.float64) * np.asarray(beta, np.float64)
    p4 = np.stack([np.asarray(conv_b, f),
                   asig.astype(f),
                   gb.astype(f),
                   np.asarray(delta, f)], axis=0)       # [4, NL, INNER]

    layer_cols = []
    for l in range(NL):
        w5l = w5[l].transpose(1, 0, 2).reshape(128, K5 * INNER)   # [d,(k i)]
        bwl = bwh[l].reshape(128, INNER)                          # [d, i]
        owl = np.ascontiguousarray(
            owh[l].transpose(2, 0, 1)).reshape(128, 2 * D)        # [i,(c d)]
        scl = np.ascontiguousarray(
            p4[:, l].reshape(4, 2, 128).transpose(2, 0, 1)).reshape(128, 8)
        layer_cols.append(np.concatenate([w5l, bwl, owl, scl], axis=1))

    oW = np.asarray(out_W, f)
    shared_cols = np.concatenate([pe8.reshape(128, 8 * 128), posb, constb]
                                 + layer_cols, axis=1)
    in_maps = []
    for r in range(NCORES):
        wpk = np.concatenate([
            x_dev[:, r * BLOC:(r + 1) * BLOC, :].reshape(128, BLOC * 512),
            shared_cols,
        ], axis=1)
        assert wpk.shape[1] == WCOLS, wpk.shape
        in_maps.append({
            "wpack": np.ascontiguousarray(wpk),
            "wt": _to_bf16(oW[r * OSL:(r + 1) * OSL].T),
        })
    return in_maps


def kernel(**inputs):
    global _PROG
    if _PROG is None:
        _PROG = build_program()
    in_maps = make_in_maps(**inputs)
    res = run_bass_kernel_spmd(_PROG, in_maps, list(range(NCORES)))
    return assemble_output([res.results[r]["y"] for r in range(NCORES)],
                           inputs["out_b"])


def assemble_output(ys, out_b=None):
    y = np.concatenate([np.asarray(yr).reshape(B, OSL) for yr in ys], axis=1)
    if out_b is not None:
        y = y + np.asarray(out_b, np.float32)[None, :]
    return y.reshape(B, C, F)


# revision 11
# speedup vs baseline: 1.2476x; 1.2476x over previous
"""Trainium2 Bass kernel for nn_CMambaSlim.

Strategy (8 NeuronCores):
  - Data-parallel trunk: each core runs the CMamba trunk (patch embed, 4
    mamba blocks, final RMSNorm) on B/8 = 4 batch samples, fp32/f32r.
  - AllGather of the flattened features (1 MB bf16) across the 8 cores.
  - Tensor-parallel output layer: core r streams rows [r*768, (r+1)*768) of
    out_W in bf16 (pre-transposed on host to [16000, 768]) and computes
    y[:, r*768:(r+1)*768]. out_b is added on the host during unsharding.

Schedule notes (CoreSim cost model):
  - All DMAs serialize on the DMA-engine device and hold the issuing
    engine's sequencer. SP's queue carries ONLY the wpack loads + the 32
    streamed weight chunks, so weight prefetch runs through the trunk and
    the AllGather. ccin/flatT/y DMAs issue from the Activation engine.
  - The residual stream h lives in PSUM: the out-projection matmuls
    accumulate straight into it (h' = h * 2^-l with the 2^-(l+1) folded
    into op_W host-side; rmsnorm is scale-invariant so only the eps
    constant needs a per-layer 4^-l).
  - The trunk is split into two independent 256-token halves (the conv
    windows are separated by a zeroed 4-column gap in hn), pipelined
    through ACT (square/sigmoid/aM), DVE (rstd/hn/u/gg/dab), Pool
    (scan/g0, SBUF-only operands), and PE.
  - ACT stays on the sigmoid table the whole trunk; rstd is computed on
    DVE as (ssum + D*eps_l)^-0.5 via AluOp.pow.
"""

import math
import os
import sys

import numpy as np

for _p in ("/opt/trn_rl_repo", "/root/.axon_site/_ro/trn_rl_repo"):
    if os.path.isdir(_p) and _p not in sys.path:
        sys.path.insert(0, _p)
        break

import concourse.bass as bass
import concourse.tile as tile
from concourse import mybir
from concourse.bass_utils import run_bass_kernel_spmd

# Model dims (hardcoded per problem spec)
B, C, L = 32, 64, 512
P, S = 16, 4
NP = 125
D = 128
INNER = 256
K5 = 5
NL = 4
F = 96
EPS = 1e-5

NCORES = 8
BLOC = B // NCORES            # 4 samples per core
OSL = (C * F) // NCORES       # 768 output cols per core
TOK = BLOC * 128              # padded token span (125 valid + 3 pad per sample)
HTOK = TOK // 2               # 256 tokens per pipelined half
HNW = 2 + HTOK + 4 + HTOK + 2  # hn with guards + inter-half gap = 520
LPAD = 520                    # x padded along L so the +8-shifted copy stays in bounds
NF = NP * D                   # 16000 contraction size
WKC = 4                       # k-chunks per weight-stream DMA
NQ = (NP + WKC - 1) // WKC    # 32 chunks (31 full + 1 partial)
WRING = 18                    # weight ring depth (chunks prefetchable)

f32 = mybir.dt.float32
f32r = mybir.dt.float32r
bf16 = mybir.dt.bfloat16
AF = mybir.ActivationFunctionType
OP = mybir.AluOpType

_PROG = None

SQRTD = math.sqrt(float(D))

# wpack layout (fp32, loaded as 5 DMAs: embed part + one per layer)
NW = 129                                   # x windows (last is zero padding)
WOFF_PE8 = BLOC * NW * 4                   # 2064 cols of x
WOFF_POSB = WOFF_PE8 + 8 * 128             # 1024 cols of patch-embed W
WOFF_IDN = WOFF_POSB + TOK                 # 512 cols: posb broadcast to (b,k)
WOFF_SQD = WOFF_IDN + 128                  # 128 cols: identity matrix
WOFF_MISC = WOFF_SQD + 128                 # 128 cols: sqrt(D) everywhere
WOFF_L0 = WOFF_MISC + 2                    # col 0: 1.0 (onesD), col 1: normf_w
LCOLS = K5 * INNER + INNER + 2 * D + 8     # 1800 cols per layer
WCOLS = WOFF_L0 + NL * LCOLS               # 11042


def build_program():
    nc = bass.Bass(num_devices=NCORES)

    wpack = nc.declare_dram_parameter("wpack", [128, WCOLS], f32, isOutput=False)
    wt = nc.declare_dram_parameter("wt", [NF, OSL], bf16, isOutput=False)
    y = nc.declare_dram_parameter("y", [B, OSL], f32, isOutput=True)

    wtT = wt[:].tensor

    with tile.TileContext(nc) as tc:
        with (
            tc.tile_pool(name="const", bufs=1) as const,
            tc.tile_pool(name="work", bufs=1) as work,
            tc.tile_pool(name="wring", bufs=WRING) as wring,
            tc.tile_pool(name="ps", bufs=1, space="PSUM") as ps,
            tc.tile_pool(name="dram", bufs=1, space="DRAM") as dram,
        ):
            # ---------------- constant loads (embed part, then per layer) ----
            wp = const.tile([128, WCOLS], f32r)
            nc.sync.dma_start(out=wp[:, 0:WOFF_L0],
                              in_=wpack[:, 0:WOFF_L0].bitcast(f32r))
            for l in range(NL):
                c0 = WOFF_L0 + l * LCOLS
                nc.sync.dma_start(out=wp[:, c0:c0 + LCOLS],
                                  in_=wpack[:, c0:c0 + LCOLS].bitcast(f32r))

            xO4 = wp[:, 0:WOFF_PE8].rearrange(
                "p (b k s) -> p b k s", b=BLOC, s=4)          # [128, 4, 129, 4]
            pe8sb = wp[:, WOFF_PE8:WOFF_POSB].rearrange("p (j d) -> p j d", j=8)
            posbB = wp[:, WOFF_POSB:WOFF_IDN]                  # [128, 512] (b,k)
            ident = wp[:, WOFF_IDN:WOFF_SQD]                   # I_128
            sqrtDrow = wp[0:1, WOFF_SQD:WOFF_SQD + 128]        # value sqrt(D)
            onesD = wp[:, WOFF_MISC:WOFF_MISC + 1]             # value 1.0
            normf = wp[:, WOFF_MISC + 1:WOFF_MISC + 2].bitcast(f32)

            def lview(l):
                b0 = WOFF_L0 + l * LCOLS
                w5 = wp[:, b0:b0 + K5 * INNER].rearrange(
                    "p (k i) -> p k i", k=K5)
                bw = wp[:, b0 + K5 * INNER:b0 + K5 * INNER + INNER]
                ow = wp[:, b0 + K5 * INNER + INNER:
                        b0 + K5 * INNER + INNER + 2 * D].rearrange(
                    "p (c d) -> p c d", c=2)
                scal = wp[:, b0 + LCOLS - 8:b0 + LCOLS].bitcast(f32).rearrange(
                    "p (s c) -> p s c", s=4)
                return w5, bw, ow, scal

            # mask01: 1 everywhere, 0 at each sample's k=0 column (scan reset)
            mask01 = const.tile([128, TOK], f32)
            nc.vector.memset(mask01[:], 1.0)
            for bq in range(BLOC):
                nc.vector.memset(mask01[:, bq * 128:bq * 128 + 1], 0.0)

            # residual stream h' lives in PSUM; out-projections accumulate
            # into it (never stopped). h' = h * 2^-l, exact via scaled op_W.
            hps = ps.tile([128, TOK], f32, tag="h", name="t_h")
            hps_bk = hps[:].rearrange("p (b k) -> p b k", b=BLOC)

            # normalized-input tile: [2 guard | half0 | 4 gap | half1 | 2 guard]
            hn = const.tile([128, HNW], f32)
            nc.vector.memset(hn[:, 0:2], 0.0)
            nc.vector.memset(hn[:, 2 + HTOK:2 + HTOK + 4], 0.0)
            nc.vector.memset(hn[:, HNW - 2:], 0.0)
            hnr = hn[:].bitcast(f32r)
            HNS = (2, 2 + HTOK + 4)        # hn write offset per half
            HR = (0, HTOK)                 # token-range start per half

            # ---------------- patch embedding (into h PSUM) ----------------
            # identity @ posbB first: start=True pending-zeros the whole
            # region and this matmul touches every byte.
            nc.tensor.matmul(out=hps[:], lhsT=ident, rhs=posbB,
                             start=True, stop=False, skip_group_check=True)
            for j in range(8):
                jq, jr = j // 4, j % 4
                rhs = xO4[:, :, jq:jq + 128, jr]
                nc.tensor.matmul(
                    out=hps[:], lhsT=pe8sb[:, j, :],
                    rhs=rhs, start=False, stop=False, skip_group_check=True)
            # zero the 3 pad tokens per sample (windows 125..127 hold junk)
            nc.vector.memset(hps_bk[:, :, 125:128], 0.0)

            # ---------------- mamba layers (two pipelined halves) -----------
            def emit_layer(l):
                w5sb, bwsb, owsb, scalsb = lview(l)
                epsl = float(D) * EPS * (0.25 ** l)
                sq = work.tile([128, TOK], f32, tag="sq", name=f"sq_{l}")
                rstd = work.tile([1, TOK], f32, tag="rstd", name=f"rstd_{l}")
                pssum = ps.tile([1, TOK], f32, tag="pss", bufs=2, name=f"pss_{l}")
                prstd = ps.tile([128, TOK], f32, tag="prstd", name=f"prstd_{l}")
                pa = [ps.tile([128, TOK], f32, tag=f"pa{ic}", name=f"pa{ic}_{l}")
                      for ic in range(2)]
                pb = [ps.tile([128, TOK], f32, tag=f"pb{ic}", name=f"pb{ic}_{l}")
                      for ic in range(2)]
                aM, sg, ab, sc, dab, gg = ({}, {}, {}, {}, {}, {})
                for ic in range(2):
                    aM[ic] = work.tile([128, TOK], f32, tag=f"am{ic}", name=f"am{ic}_{l}")
                    sg[ic] = work.tile([128, TOK], f32, tag=f"sg{ic}", name=f"sg{ic}_{l}")
                    ab[ic] = work.tile([128, TOK], f32, tag=f"ab{ic}", name=f"ab{ic}_{l}")
                    sc[ic] = work.tile([128, TOK], f32, tag=f"s{ic}", name=f"s{ic}_{l}")
                    dab[ic] = work.tile([128, TOK], f32, tag=f"dab{ic}", name=f"dab{ic}_{l}")
                    gg[ic] = work.tile([128, TOK], f32, tag=f"g{ic}", name=f"g{ic}_{l}")

                for hh in range(2):
                    r0 = HR[hh]
                    nc.scalar.activation(out=sq[:, r0:r0 + HTOK],
                                         in_=hps[:, r0:r0 + HTOK], func=AF.Square)
                for hh in range(2):
                    r0 = HR[hh]
                    nc.tensor.matmul(
                        out=pssum[0:1, r0:r0 + HTOK], lhsT=onesD,
                        rhs=sq[:, r0:r0 + HTOK].bitcast(f32r),
                        start=True, stop=True, skip_group_check=True)
                for hh in range(2):
                    r0 = HR[hh]
                    nc.vector.tensor_scalar(
                        out=rstd[0:1, r0:r0 + HTOK], in0=pssum[0:1, r0:r0 + HTOK],
                        scalar1=epsl, scalar2=-0.5, op0=OP.add, op1=OP.pow)
                for hh in range(2):
                    r0 = HR[hh]
                    nc.tensor.matmul(
                        out=prstd[:, r0:r0 + HTOK], lhsT=sqrtDrow,
                        rhs=rstd[0:1, r0:r0 + HTOK].bitcast(f32r),
                        start=True, stop=True, skip_group_check=True)
                for hh in range(2):
                    r0, h0 = HR[hh], HNS[hh]
                    nc.vector.tensor_tensor(
                        out=hn[:, h0:h0 + HTOK], in0=hps[:, r0:r0 + HTOK],
                        in1=prstd[:, r0:r0 + HTOK], op=OP.mult)
                for hh in range(2):
                    r0 = HR[hh]
                    for ic in range(2):
                        nc.scalar.activation(
                            out=aM[ic][:, r0:r0 + HTOK], in_=mask01[:, r0:r0 + HTOK],
                            func=AF.Copy, scale=scalsb[:, 1, ic:ic + 1])
                for hh in range(2):
                    h0 = HNS[hh]
                    for ic in range(2):
                        for dk in range(K5):
                            nc.tensor.matmul(
                                out=pa[ic][:, HR[hh]:HR[hh] + HTOK],
                                lhsT=w5sb[:, dk, ic * 128:(ic + 1) * 128],
                                rhs=hnr[:, h0 - 2 + dk:h0 - 2 + dk + HTOK],
                                start=(dk == 0), stop=(dk == K5 - 1),
                                skip_group_check=True)
                for hh in range(2):
                    h0 = HNS[hh]
                    for ic in range(2):
                        nc.tensor.matmul(
                            out=pb[ic][:, HR[hh]:HR[hh] + HTOK],
                            lhsT=bwsb[:, ic * 128:(ic + 1) * 128],
                            rhs=hnr[:, h0:h0 + HTOK],
                            start=True, stop=True, skip_group_check=True)
                for hh in range(2):
                    r0 = HR[hh]
                    for ic in range(2):
                        # silu(z) = z * sigmoid(z), z = conv + conv_b
                        nc.scalar.activation(
                            out=sg[ic][:, r0:r0 + HTOK], in_=pa[ic][:, r0:r0 + HTOK],
                            func=AF.Sigmoid, bias=scalsb[:, 0, ic:ic + 1], scale=1.0)
                for hh in range(2):
                    r0 = HR[hh]
                    for ic in range(2):
                        nc.vector.scalar_tensor_tensor(
                            out=ab[ic][:, r0:r0 + HTOK], in0=pa[ic][:, r0:r0 + HTOK],
                            scalar=scalsb[:, 0, ic:ic + 1],
                            in1=sg[ic][:, r0:r0 + HTOK], op0=OP.add, op1=OP.mult)
                for hh in range(2):
                    r0 = HR[hh]
                    for ic in range(2):
                        # scan: state = aM*state + u (Pool; SBUF operands only)
                        nc.gpsimd.tensor_tensor_scan(
                            out=sc[ic][:, r0:r0 + HTOK], data0=aM[ic][:, r0:r0 + HTOK],
                            data1=ab[ic][:, r0:r0 + HTOK], initial=0.0,
                            op0=OP.mult, op1=OP.add)
                for hh in range(2):
                    r0 = HR[hh]
                    for ic in range(2):
                        nc.vector.tensor_scalar_mul(
                            out=dab[ic][:, r0:r0 + HTOK], in0=ab[ic][:, r0:r0 + HTOK],
                            scalar1=scalsb[:, 3, ic:ic + 1])
                for hh in range(2):
                    r0 = HR[hh]
                    for ic in range(2):
                        # g0 = gamma*beta*s + dab (Pool; SBUF operands only)
                        nc.gpsimd.scalar_tensor_tensor(
                            out=sc[ic][:, r0:r0 + HTOK], in0=sc[ic][:, r0:r0 + HTOK],
                            scalar=scalsb[:, 2, ic:ic + 1],
                            in1=dab[ic][:, r0:r0 + HTOK], op0=OP.mult, op1=OP.add)
                for hh in range(2):
                    r0 = HR[hh]
                    for ic in range(2):
                        # pads stay zero: hn pads are zero so pb pads are zero
                        nc.vector.tensor_tensor(
                            out=gg[ic][:, r0:r0 + HTOK], in0=sc[ic][:, r0:r0 + HTOK],
                            in1=pb[ic][:, r0:r0 + HTOK], op=OP.mult)
                for hh in range(2):
                    r0 = HR[hh]
                    for ic in range(2):
                        # residual: h' += 2^-(l+1) * oW @ g (scale folded into oW)
                        nc.tensor.matmul(
                            out=hps[:, r0:r0 + HTOK], lhsT=owsb[:, ic, :],
                            rhs=gg[ic][:, r0:r0 + HTOK].bitcast(f32r),
                            start=False, stop=False, skip_group_check=True)

            for l in range(NL):
                emit_layer(l)

            # ---------------- final rmsnorm ----------------
            epsf = float(D) * EPS * (0.25 ** NL)
            sqf = work.tile([128, TOK], f32, tag="sq", name="t_sqf")
            rstdf = work.tile([1, TOK], f32, tag="rstd", name="t_rstdf")
            pssumf = ps.tile([1, TOK], f32, tag="pss", bufs=2, name="t_pssf")
            prstdf = ps.tile([128, TOK], f32, tag="prstd", name="t_prstdf")
            hf = work.tile([128, TOK], bf16, tag="hf", name="t_hf")
            for hh in range(2):
                r0 = HR[hh]
                nc.scalar.activation(out=sqf[:, r0:r0 + HTOK],
                                     in_=hps[:, r0:r0 + HTOK], func=AF.Square)
            for hh in range(2):
                r0 = HR[hh]
                nc.tensor.matmul(
                    out=pssumf[0:1, r0:r0 + HTOK], lhsT=onesD,
                    rhs=sqf[:, r0:r0 + HTOK].bitcast(f32r),
                    start=True, stop=True, skip_group_check=True)
            for hh in range(2):
                r0 = HR[hh]
                nc.vector.tensor_scalar(
                    out=rstdf[0:1, r0:r0 + HTOK], in0=pssumf[0:1, r0:r0 + HTOK],
                    scalar1=epsf, scalar2=-0.5, op0=OP.add, op1=OP.pow)
            for hh in range(2):
                r0 = HR[hh]
                nc.tensor.matmul(
                    out=prstdf[:, r0:r0 + HTOK], lhsT=sqrtDrow,
                    rhs=rstdf[0:1, r0:r0 + HTOK].bitcast(f32r),
                    start=True, stop=True, skip_group_check=True)
            for hh in range(2):
                r0 = HR[hh]
                nc.vector.scalar_tensor_tensor(
                    out=hf[:, r0:r0 + HTOK], in0=hps[:, r0:r0 + HTOK],
                    scalar=normf, in1=prstdf[:, r0:r0 + HTOK],
                    op0=OP.mult, op1=OP.mult)

            # ---------------- all-gather the features (bf16) ----------------
            ccin = dram.tile([128, TOK], bf16)
            nc.scalar.dma_start(out=ccin[:], in_=hf[:])
            # inner dim padded so the gathered blocks stay stride-separated
            TOKP = TOK + 8
            ccout = dram.tile([NCORES, 128, TOKP], bf16, addr_space="Shared")
            nc.gpsimd.collective_compute(
                "AllGather", OP.bypass,
                replica_groups=[list(range(NCORES))],
                ins=[ccin[:].opt()], outs=[ccout[:, :, 0:TOK]])
            # flatT[d, b, k] (k padded to 128; pads are zero), b = r*BLOC + b4
            flatT = const.tile([128, B, 128], bf16)
            nc.scalar.dma_start(
                out=flatT[:].rearrange("p (r x) k -> p r (x k)", r=NCORES),
                in_=bass.AP(tensor=ccout[:].tensor, offset=ccout[:].offset,
                            ap=[[TOKP, 128], [128 * TOKP, NCORES], [1, TOK]]),
            )
            fap = flatT[:]
            fp0 = list(fap.ap[0])

            # ---------------- streamed output matmul ----------------
            # y[b, o] accumulated over the 125 (k, d) chunks. Stationary
            # operand = flatT columns (j, b) at offset k: output row j*32+b
            # holds sum_d flat[d, b, k+j] * wt_k[d, o]; rows 0..31 / j=0 are
            # the real batch rows, the rest M-padding. Moving operand = the
            # streamed bf16 W tile.
            pyb0 = ps.tile([128, 512], f32, tag="pa0", name="t_pyb0")
            pyb1 = ps.tile([128, OSL - 512], f32, tag="pa1", name="t_pyb1")
            pybs = ((pyb0, 0, 512), (pyb1, 512, OSL - 512))
            for q in range(NQ):
                kc_n = min(WKC, NP - q * WKC)
                wtl = wring.tile([128, WKC, OSL], bf16, tag="wt", name="t_wt")
                nc.sync.dma_start(
                    out=wtl[:, 0:kc_n, :],
                    in_=bass.AP(tensor=wtT, offset=q * WKC * 128 * OSL,
                                ap=[[OSL, 128], [128 * OSL, kc_n], [1, OSL]]),
                )
                for kc in range(kc_n):
                    k = q * WKC + kc
                    lhsT = bass.AP(tensor=fap.tensor, offset=fap.offset + k,
                                   ap=[fp0, [1, 4], [128, 32]])
                    for (pt, o0, on) in pybs:
                        nc.tensor.matmul(
                            out=pt[:], lhsT=lhsT,
                            rhs=wtl[:, kc, o0:o0 + on],
                            start=(k == 0), stop=(k == NP - 1),
                            skip_group_check=True)

            yout = work.tile([32, OSL], f32, tag="yout", name="t_yout")
            nc.scalar.copy(out=yout[:, 0:512], in_=pyb0[0:32, :])
            nc.scalar.copy(out=yout[:, 512:OSL], in_=pyb1[0:32, :])
            nc.scalar.dma_start(out=y[:], in_=yout[:])

    _legalize_waits(nc)
    return nc


def _legalize_waits(nc):
    """walrus on this toolchain accepts only one sync wait per non-sequencer
    instruction. Move extra waits onto standalone InstEventSemaphore
    instructions (sequencer-level waits, multi-wait legal) placed just
    before the owning instruction on the same engine."""
    n_moved = 0
    for bb in nc.main_func.blocks:
        out = []
        for inst in bb.instructions:
            si = inst.sync_info
            tn = type(inst).__name__
            if (si is not None and len(si.on_wait) > 1
                    and tn not in ("InstEventSemaphore", "InstNoOp")):
                waits = list(si.on_wait)
                for w in waits[:-1]:
                    ev = mybir.InstNoOp(
                        name=f"lw_{inst.name}_{n_moved}", ins=[], outs=[],
                        engine=inst.engine)
                    ev.sync_info = mybir.SyncInfo(on_wait=[w], on_update=[])
                    nc.register_instruction(ev)
                    out.append(ev)
                    n_moved += 1
                inst.sync_info = mybir.SyncInfo(
                    on_wait=[waits[-1]], on_update=list(si.on_update))
            out.append(inst)
        bb.instructions = out


def _sincos_pe(n, d):
    pos = np.arange(n, dtype=np.float32)[:, None]
    sin_cols, cos_cols = (d + 1) // 2, d // 2
    denom = d / 2.0
    sin_div = np.exp(
        (-math.log(10000.0) * np.arange(sin_cols, dtype=np.float32) / denom)
    ).astype(np.float32)
    cos_div = np.exp(
        (-math.log(10000.0) * np.arange(cos_cols, dtype=np.float32) / denom)
    ).astype(np.float32)
    pe = np.zeros((n, d), dtype=np.float32)
    pe[:, 0::2] = np.sin(pos * sin_div[None, :])
    pe[:, 1::2] = np.cos(pos * cos_div[None, :])
    return pe


def _to_bf16(a):
    import ml_dtypes
    return np.asarray(a, np.float32).astype(ml_dtypes.bfloat16)


def make_in_maps(x, pe_W, pe_b, norm_w, ipa_W, ipb_W, conv_W, conv_b,
                 alpha, beta, gamma, delta, op_W, normf_w, out_W, out_b):
    f = np.float32
    x = np.asarray(x, f)
    x_pad = np.zeros((B, C, LPAD + 4), f)
    x_pad[:, :, :L] = x
    # device layout: [p2*64+c, b_loc, l] with p2=1 rows shifted by 8 along l;
    # 129 windows of 4 (the last is zero padding for the shifted matmuls)
    xcT = x_pad.transpose(1, 0, 2)                     # [c, b, lpad]
    x_dev = np.empty((2, C, B, NW * 4), f)
    x_dev[0] = xcT[:, :, 0:NW * 4]
    x_dev[1] = xcT[:, :, 8:8 + NW * 4]
    x_dev = x_dev.reshape(128, B, NW * 4)

    pw = np.asarray(pe_W, f).reshape(D, C, P)          # [d, c, p]
    t = pw.transpose(1, 2, 0)                          # [c, p, d]
    pe8 = np.ascontiguousarray(
        t.reshape(C, 2, 8, D).transpose(2, 1, 0, 3).reshape(8, 128, 128))
    pe8 = np.ascontiguousarray(pe8.transpose(1, 0, 2))  # [pp, j, d]

    posb = _sincos_pe(NP, D).T + np.asarray(pe_b, f)[:, None]   # [128, 125]
    posbB = np.zeros((128, BLOC, 128), f)
    posbB[:, :, :NP] = posb[:, None, :]
    posbB = posbB.reshape(128, TOK)

    ident = np.eye(128, dtype=f)
    sqd = np.full((128, 128), SQRTD, f)
    misc = np.zeros((128, 2), f)
    misc[:, 0] = 1.0
    misc[:, 1] = np.asarray(normf_w, f)

    nw = np.asarray(norm_w, f)                          # [NL, D]
    ipa = np.asarray(ipa_W, f)                          # [NL, INNER, D]
    cw = np.asarray(conv_W, f)[:, :, 0, :]              # [NL, INNER, K5]
    w5 = (ipa.transpose(0, 2, 1)[:, None, :, :]         # [NL, 1, D, INNER]
          * cw.transpose(0, 2, 1)[:, :, None, :]        # [NL, K5, 1, INNER]
          * nw[:, None, :, None])                       # [NL, K5, D, INNER]
    bwh = np.asarray(ipb_W, f).transpose(0, 2, 1) * nw[:, :, None]  # [NL, D, INNER]
    owh = np.asarray(op_W, f).transpose(0, 2, 1).reshape(NL, 2, 128, D)

    asig = 1.0 / (1.0 + np.exp(-np.asarray(alpha, np.float64)))
    gb = np.asarray(gamma, np.float64) * np.asarray(beta, np.float64)
    p4 = np.stack([np.asarray(conv_b, f),
                   asig.astype(f),
                   gb.astype(f),
                   np.asarray(delta, f)], axis=0)       # [4, NL, INNER]

    layer_cols = []
    for l in range(NL):
        w5l = w5[l].transpose(1, 0, 2).reshape(128, K5 * INNER)   # [d,(k i)]
        bwl = bwh[l].reshape(128, INNER)                          # [d, i]
        # residual lives in PSUM as h' = h * 2^-l: fold 2^-(l+1) into op_W
        owl = np.ascontiguousarray(
            owh[l].transpose(2, 0, 1)).reshape(128, 2 * D) * (0.5 ** (l + 1))
        scl = np.ascontiguousarray(
            p4[:, l].reshape(4, 2, 128).transpose(2, 0, 1)).reshape(128, 8)
        layer_cols.append(np.concatenate(
            [w5l, bwl, owl.astype(f), scl], axis=1))

    oW = np.asarray(out_W, f)
    shared_cols = np.concatenate(
        [pe8.reshape(128, 8 * 128), posbB, ident, sqd, misc] + layer_cols,
        axis=1)
    in_maps = []
    for r in range(NCORES):
        wpk = np.concatenate([
            x_dev[:, r * BLOC:(r + 1) * BLOC, :].reshape(128, BLOC * NW * 4),
            shared_cols,
        ], axis=1)
        assert wpk.shape[1] == WCOLS, wpk.shape
        in_maps.append({
            "wpack": np.ascontiguousarray(wpk),
            "wt": _to_bf16(oW[r * OSL:(r + 1) * OSL].T),
        })
    return in_maps


def kernel(**inputs):
    global _PROG
    if _PROG is None:
        _PROG = build_program()
    in_maps = make_in_maps(**inputs)
    res = run_bass_kernel_spmd(_PROG, in_maps, list(range(NCORES)))
    return assemble_output([res.results[r]["y"] for r in range(NCORES)],
                           inputs["out_b"])


def assemble_output(ys, out_b=None):
    y = np.concatenate([np.asarray(yr).reshape(B, OSL) for yr in ys], axis=1)
    if out_b is not None:
        y = y + np.asarray(out_b, np.float32)[None, :]
    return y.reshape(B, C, F)


# revision 18
# speedup vs baseline: 1.2583x; 1.0086x over previous
"""Trainium2 Bass kernel for nn_CMambaSlim.

Strategy (8 NeuronCores):
  - Data-parallel trunk: each core runs the CMamba trunk (patch embed, 4
    mamba blocks, final RMSNorm) on B/8 = 4 batch samples, fp32/f32r.
  - AllGather of the flattened features (1 MB bf16) across the 8 cores.
  - Tensor-parallel output layer: core r streams rows [r*768, (r+1)*768) of
    out_W in bf16 (pre-transposed on host to [16000, 768]) and computes
    y[:, r*768:(r+1)*768]. out_b is added on the host during unsharding.

Schedule notes (CoreSim cost model):
  - All DMAs serialize on the DMA-engine device and hold the issuing
    engine's sequencer. SP's queue carries ONLY the wpack loads + the 32
    streamed weight chunks, so weight prefetch runs through the trunk and
    the AllGather. ccin/flatT/y DMAs issue from the Activation engine.
  - The residual stream h lives in PSUM: the out-projection matmuls
    accumulate straight into it (h' = h * 2^-l with the 2^-(l+1) folded
    into op_W host-side; rmsnorm is scale-invariant so only the eps
    constant needs a per-layer 4^-l).
  - The trunk is split into two independent 256-token halves (the conv
    windows are separated by a zeroed 4-column gap in hn), pipelined
    through ACT (square/sigmoid/aM), DVE (rstd/hn/u/gg/dab), Pool
    (scan/g0, SBUF-only operands), and PE.
  - ACT stays on the sigmoid table the whole trunk; rstd is computed on
    DVE as (ssum + D*eps_l)^-0.5 via AluOp.pow.
"""

import math
import os
import sys

import numpy as np

for _p in ("/opt/trn_rl_repo", "/root/.axon_site/_ro/trn_rl_repo"):
    if os.path.isdir(_p) and _p not in sys.path:
        sys.path.insert(0, _p)
        break

import concourse.bass as bass
import concourse.tile as tile
from concourse import mybir
from concourse.bass_utils import run_bass_kernel_spmd

# Model dims (hardcoded per problem spec)
B, C, L = 32, 64, 512
P, S = 16, 4
NP = 125
D = 128
INNER = 256
K5 = 5
NL = 4
F = 96
EPS = 1e-5

NCORES = 8
BLOC = B // NCORES            # 4 samples per core
OSL = (C * F) // NCORES       # 768 output cols per core
TOK = BLOC * 128              # padded token span (125 valid + 3 pad per sample)
HTOK = TOK // 2               # 256 tokens per pipelined half
HNW = 2 + HTOK + 4 + HTOK + 2  # hn with guards + inter-half gap = 520
LPAD = 520                    # x padded along L so the +8-shifted copy stays in bounds
NF = NP * D                   # 16000 contraction size
WKC = 4                       # k-chunks per weight-stream DMA
NQ = (NP + WKC - 1) // WKC    # 32 chunks (31 full + 1 partial)
WRING = 18                    # weight ring depth (chunks prefetchable)

f32 = mybir.dt.float32
f32r = mybir.dt.float32r
bf16 = mybir.dt.bfloat16
AF = mybir.ActivationFunctionType
OP = mybir.AluOpType

_PROG = None

SQRTD = math.sqrt(float(D))

# wph: bf16 embed inputs (x windows + patch-embed W), loaded first
NW = 129                                   # x windows (last is zero padding)
HOFF_PE8 = BLOC * NW * 4                   # 2064 cols of x
HCOLS = HOFF_PE8 + 8 * 128                 # + 1024 cols of patch-embed W
# wpack: fp32 consts (embed tail + one DMA per layer)
WOFF_IDN = TOK                             # 512 cols: posb broadcast to (b,k)
WOFF_SQD = WOFF_IDN + 128                  # 128 cols: identity matrix
WOFF_MISC = WOFF_SQD + 128                 # 128 cols: sqrt(D) everywhere
WOFF_L0 = WOFF_MISC + 2                    # col 0: 1.0 (onesD), col 1: normf_w
LCOLS = K5 * INNER + INNER + 2 * D + 8     # 1800 cols per layer
WCOLS = WOFF_L0 + NL * LCOLS               # 7970


def build_program():
    nc = bass.Bass(num_devices=NCORES)

    wph = nc.declare_dram_parameter("wph", [128, HCOLS], bf16, isOutput=False)
    wpack = nc.declare_dram_parameter("wpack", [128, WCOLS], f32, isOutput=False)
    wt = nc.declare_dram_parameter("wt", [NF, OSL], bf16, isOutput=False)
    y = nc.declare_dram_parameter("y", [B, OSL], f32, isOutput=True)

    wtT = wt[:].tensor

    with tile.TileContext(nc) as tc:
        with (
            tc.tile_pool(name="const", bufs=1) as const,
            tc.tile_pool(name="work", bufs=1) as work,
            tc.tile_pool(name="wring", bufs=WRING) as wring,
            tc.tile_pool(name="ps", bufs=1, space="PSUM") as ps,
            tc.tile_pool(name="dram", bufs=1, space="DRAM") as dram,
        ):
            # ---------------- constant loads (embed parts, then per layer) ---
            wh = const.tile([128, HCOLS], bf16)
            nc.sync.dma_start(out=wh[:], in_=wph[:])
            wp = const.tile([128, WCOLS], f32r)
            nc.sync.dma_start(out=wp[:, 0:WOFF_L0],
                              in_=wpack[:, 0:WOFF_L0].bitcast(f32r))
            for l in range(NL):
                c0 = WOFF_L0 + l * LCOLS
                nc.sync.dma_start(out=wp[:, c0:c0 + LCOLS],
                                  in_=wpack[:, c0:c0 + LCOLS].bitcast(f32r))

            xO4 = wh[:, 0:HOFF_PE8].rearrange(
                "p (b k s) -> p b k s", b=BLOC, s=4)          # [128, 4, 129, 4]
            pe8sb = wh[:, HOFF_PE8:HCOLS].rearrange("p (j d) -> p j d", j=8)
            posbB = wp[:, 0:WOFF_IDN]                          # [128, 512] (b,k)
            ident = wp[:, WOFF_IDN:WOFF_SQD]                   # I_128
            sqrtDrow = wp[0:1, WOFF_SQD:WOFF_SQD + 128]        # value sqrt(D)
            onesD = wp[:, WOFF_MISC:WOFF_MISC + 1]             # value 1.0
            normf = wp[:, WOFF_MISC + 1:WOFF_MISC + 2].bitcast(f32)

            def lview(l):
                b0 = WOFF_L0 + l * LCOLS
                w5 = wp[:, b0:b0 + K5 * INNER].rearrange(
                    "p (k i) -> p k i", k=K5)
                bw = wp[:, b0 + K5 * INNER:b0 + K5 * INNER + INNER]
                ow = wp[:, b0 + K5 * INNER + INNER:
                        b0 + K5 * INNER + INNER + 2 * D].rearrange(
                    "p (c d) -> p c d", c=2)
                scal = wp[:, b0 + LCOLS - 8:b0 + LCOLS].bitcast(f32).rearrange(
                    "p (s c) -> p s c", s=4)
                return w5, bw, ow, scal

            # mask01: 1 everywhere, 0 at each sample's k=0 column (scan reset)
            mask01 = const.tile([128, TOK], f32)
            nc.vector.memset(mask01[:], 1.0)
            for bq in range(BLOC):
                nc.vector.memset(mask01[:, bq * 128:bq * 128 + 1], 0.0)

            # residual stream h' lives in PSUM; out-projections accumulate
            # into it (never stopped). h' = h * 2^-l, exact via scaled op_W.
            hps = ps.tile([128, TOK], f32, tag="h", name="t_h")
            hps_bk = hps[:].rearrange("p (b k) -> p b k", b=BLOC)

            # normalized-input tile: [2 guard | half0 | 4 gap | half1 | 2 guard]
            hn = const.tile([128, HNW], f32)
            nc.vector.memset(hn[:, 0:2], 0.0)
            nc.vector.memset(hn[:, 2 + HTOK:2 + HTOK + 4], 0.0)
            nc.vector.memset(hn[:, HNW - 2:], 0.0)
            hnr = hn[:].bitcast(f32r)
            HNS = (2, 2 + HTOK + 4)        # hn write offset per half
            HR = (0, HTOK)                 # token-range start per half

            # ---------------- patch embedding (into h PSUM) ----------------
            # identity @ posbB first: start=True pending-zeros the whole
            # region and this matmul touches every byte.
            nc.tensor.matmul(out=hps[:], lhsT=ident, rhs=posbB,
                             start=True, stop=False, skip_group_check=True)
            for j in range(8):
                jq, jr = j // 4, j % 4
                rhs = xO4[:, :, jq:jq + 128, jr]
                nc.tensor.matmul(
                    out=hps[:], lhsT=pe8sb[:, j, :],
                    rhs=rhs, start=False, stop=False, skip_group_check=True)
            # zero the 3 pad tokens per sample (windows 125..127 hold junk)
            nc.vector.memset(hps_bk[:, :, 125:128], 0.0)

            # ---------------- mamba layers (two pipelined halves) -----------
            def emit_layer(l):
                w5sb, bwsb, owsb, scalsb = lview(l)
                epsl = float(D) * EPS * (0.25 ** l)
                sq = work.tile([128, TOK], f32, tag="sq", name=f"sq_{l}")
                rstd = work.tile([1, TOK], f32, tag="rstd", name=f"rstd_{l}")
                pssum = ps.tile([1, TOK], f32, tag="pss", bufs=2, name=f"pss_{l}")
                prstd = ps.tile([128, TOK], f32, tag="prstd", name=f"prstd_{l}")
                pa = [ps.tile([128, TOK], f32, tag=f"pa{ic}", name=f"pa{ic}_{l}")
                      for ic in range(2)]
                pb = [ps.tile([128, TOK], f32, tag=f"pb{ic}", name=f"pb{ic}_{l}")
                      for ic in range(2)]
                aM, sg, ab, sc, dab, gg = ({}, {}, {}, {}, {}, {})
                for ic in range(2):
                    aM[ic] = work.tile([128, TOK], f32, tag=f"am{ic}", bufs=2,
                                       name=f"am{ic}_{l}")
                    sg[ic] = work.tile([128, TOK], f32, tag=f"sg{ic}", name=f"sg{ic}_{l}")
                    ab[ic] = work.tile([128, TOK], f32, tag=f"ab{ic}", name=f"ab{ic}_{l}")
                    sc[ic] = work.tile([128, TOK], f32, tag=f"s{ic}", name=f"s{ic}_{l}")
                    dab[ic] = work.tile([128, TOK], f32, tag=f"dab{ic}", name=f"dab{ic}_{l}")
                    gg[ic] = work.tile([128, TOK], f32, tag=f"g{ic}", name=f"g{ic}_{l}")

                # aM first: depends only on constants, fills ACT while the
                # previous layer's gate phase runs (bufs=2 on the am tags)
                for hh in range(2):
                    r0 = HR[hh]
                    for ic in range(2):
                        nc.scalar.activation(
                            out=aM[ic][:, r0:r0 + HTOK], in_=mask01[:, r0:r0 + HTOK],
                            func=AF.Copy, scale=scalsb[:, 1, ic:ic + 1])
                for hh in range(2):
                    r0 = HR[hh]
                    nc.scalar.activation(out=sq[:, r0:r0 + HTOK],
                                         in_=hps[:, r0:r0 + HTOK], func=AF.Square)
                for hh in range(2):
                    r0 = HR[hh]
                    nc.tensor.matmul(
                        out=pssum[0:1, r0:r0 + HTOK], lhsT=onesD,
                        rhs=sq[:, r0:r0 + HTOK].bitcast(f32r),
                        start=True, stop=True, skip_group_check=True)
                for hh in range(2):
                    r0 = HR[hh]
                    nc.vector.tensor_scalar(
                        out=rstd[0:1, r0:r0 + HTOK], in0=pssum[0:1, r0:r0 + HTOK],
                        scalar1=epsl, scalar2=-0.5, op0=OP.add, op1=OP.pow)
                for hh in range(2):
                    r0 = HR[hh]
                    nc.tensor.matmul(
                        out=prstd[:, r0:r0 + HTOK], lhsT=sqrtDrow,
                        rhs=rstd[0:1, r0:r0 + HTOK].bitcast(f32r),
                        start=True, stop=True, skip_group_check=True)
                for hh in range(2):
                    r0, h0 = HR[hh], HNS[hh]
                    nc.vector.tensor_tensor(
                        out=hn[:, h0:h0 + HTOK], in0=hps[:, r0:r0 + HTOK],
                        in1=prstd[:, r0:r0 + HTOK], op=OP.mult)
                for hh in range(2):
                    h0 = HNS[hh]
                    for ic in range(2):
                        for dk in range(K5):
                            nc.tensor.matmul(
                                out=pa[ic][:, HR[hh]:HR[hh] + HTOK],
                                lhsT=w5sb[:, dk, ic * 128:(ic + 1) * 128],
                                rhs=hnr[:, h0 - 2 + dk:h0 - 2 + dk + HTOK],
                                start=(dk == 0), stop=(dk == K5 - 1),
                                skip_group_check=True)
                    for ic in range(2):
                        nc.tensor.matmul(
                            out=pb[ic][:, HR[hh]:HR[hh] + HTOK],
                            lhsT=bwsb[:, ic * 128:(ic + 1) * 128],
                            rhs=hnr[:, h0:h0 + HTOK],
                            start=True, stop=True, skip_group_check=True)
                # gate phase: half-major so the two halves pipeline cleanly
                for hh in range(2):
                    r0 = HR[hh]
                    for ic in range(2):
                        # silu(z) = z * sigmoid(z), z = conv + conv_b
                        nc.scalar.activation(
                            out=sg[ic][:, r0:r0 + HTOK], in_=pa[ic][:, r0:r0 + HTOK],
                            func=AF.Sigmoid, bias=scalsb[:, 0, ic:ic + 1], scale=1.0)
                    for ic in range(2):
                        nc.vector.scalar_tensor_tensor(
                            out=ab[ic][:, r0:r0 + HTOK], in0=pa[ic][:, r0:r0 + HTOK],
                            scalar=scalsb[:, 0, ic:ic + 1],
                            in1=sg[ic][:, r0:r0 + HTOK], op0=OP.add, op1=OP.mult)
                    for ic in range(2):
                        # scan: state = aM*state + u (Pool; SBUF operands only)
                        nc.gpsimd.tensor_tensor_scan(
                            out=sc[ic][:, r0:r0 + HTOK], data0=aM[ic][:, r0:r0 + HTOK],
                            data1=ab[ic][:, r0:r0 + HTOK], initial=0.0,
                            op0=OP.mult, op1=OP.add)
                    for ic in range(2):
                        nc.vector.tensor_scalar_mul(
                            out=dab[ic][:, r0:r0 + HTOK], in0=ab[ic][:, r0:r0 + HTOK],
                            scalar1=scalsb[:, 3, ic:ic + 1])
                    for ic in range(2):
                        # g0 = gamma*beta*s + dab (Pool; SBUF operands only)
                        nc.gpsimd.scalar_tensor_tensor(
                            out=sc[ic][:, r0:r0 + HTOK], in0=sc[ic][:, r0:r0 + HTOK],
                            scalar=scalsb[:, 2, ic:ic + 1],
                            in1=dab[ic][:, r0:r0 + HTOK], op0=OP.mult, op1=OP.add)
                    for ic in range(2):
                        # pads stay zero: hn pads are zero so pb pads are zero
                        nc.vector.tensor_tensor(
                            out=gg[ic][:, r0:r0 + HTOK], in0=sc[ic][:, r0:r0 + HTOK],
                            in1=pb[ic][:, r0:r0 + HTOK], op=OP.mult)
                    for ic in range(2):
                        # residual: h' += 2^-(l+1) * oW @ g (scale folded into oW)
                        nc.tensor.matmul(
                            out=hps[:, r0:r0 + HTOK], lhsT=owsb[:, ic, :],
                            rhs=gg[ic][:, r0:r0 + HTOK].bitcast(f32r),
                            start=False, stop=False, skip_group_check=True)

            for l in range(NL):
                emit_layer(l)

            # ---------------- final rmsnorm ----------------
            epsf = float(D) * EPS * (0.25 ** NL)
            sqf = work.tile([128, TOK], f32, tag="sq", name="t_sqf")
            rstdf = work.tile([1, TOK], f32, tag="rstd", name="t_rstdf")
            pssumf = ps.tile([1, TOK], f32, tag="pss", bufs=2, name="t_pssf")
            prstdf = ps.tile([128, TOK], f32, tag="prstd", name="t_prstdf")
            hf = work.tile([128, TOK], bf16, tag="hf", name="t_hf")
            for hh in range(2):
                r0 = HR[hh]
                nc.scalar.activation(out=sqf[:, r0:r0 + HTOK],
                                     in_=hps[:, r0:r0 + HTOK], func=AF.Square)
            for hh in range(2):
                r0 = HR[hh]
                nc.tensor.matmul(
                    out=pssumf[0:1, r0:r0 + HTOK], lhsT=onesD,
                    rhs=sqf[:, r0:r0 + HTOK].bitcast(f32r),
                    start=True, stop=True, skip_group_check=True)
            for hh in range(2):
                r0 = HR[hh]
                nc.vector.tensor_scalar(
                    out=rstdf[0:1, r0:r0 + HTOK], in0=pssumf[0:1, r0:r0 + HTOK],
                    scalar1=epsf, scalar2=-0.5, op0=OP.add, op1=OP.pow)
            for hh in range(2):
                r0 = HR[hh]
                nc.tensor.matmul(
                    out=prstdf[:, r0:r0 + HTOK], lhsT=sqrtDrow,
                    rhs=rstdf[0:1, r0:r0 + HTOK].bitcast(f32r),
                    start=True, stop=True, skip_group_check=True)
            for hh in range(2):
                r0 = HR[hh]
                nc.vector.scalar_tensor_tensor(
                    out=hf[:, r0:r0 + HTOK], in0=hps[:, r0:r0 + HTOK],
                    scalar=normf, in1=prstdf[:, r0:r0 + HTOK],
                    op0=OP.mult, op1=OP.mult)

            # ---------------- all-gather the features (bf16) ----------------
            ccin = dram.tile([128, TOK], bf16)
            nc.scalar.dma_start(out=ccin[:], in_=hf[:])
            # inner dim padded so the gathered blocks stay stride-separated
            TOKP = TOK + 8
            ccout = dram.tile([NCORES, 128, TOKP], bf16, addr_space="Shared")
            nc.gpsimd.collective_compute(
                "AllGather", OP.bypass,
                replica_groups=[list(range(NCORES))],
                ins=[ccin[:].opt()], outs=[ccout[:, :, 0:TOK]])
            # flatT[d, b, k] (k padded to 128; pads are zero), b = r*BLOC + b4
            flatT = const.tile([128, B, 128], bf16)
            nc.scalar.dma_start(
                out=flatT[:].rearrange("p (r x) k -> p r (x k)", r=NCORES),
                in_=bass.AP(tensor=ccout[:].tensor, offset=ccout[:].offset,
                            ap=[[TOKP, 128], [128 * TOKP, NCORES], [1, TOK]]),
            )
            fap = flatT[:]
            fp0 = list(fap.ap[0])

            # ---------------- streamed output matmul ----------------
            # y[b, o] accumulated over the 125 (k, d) chunks. Stationary
            # operand = flatT columns (j, b) at offset k: output row j*32+b
            # holds sum_d flat[d, b, k+j] * wt_k[d, o]; rows 0..31 / j=0 are
            # the real batch rows, the rest M-padding. Moving operand = the
            # streamed bf16 W tile.
            pyb0 = ps.tile([128, 512], f32, tag="pa0", name="t_pyb0")
            pyb1 = ps.tile([128, OSL - 512], f32, tag="pa1", name="t_pyb1")
            pybs = ((pyb0, 0, 512), (pyb1, 512, OSL - 512))
            for q in range(NQ):
                kc_n = min(WKC, NP - q * WKC)
                wtl = wring.tile([128, WKC, OSL], bf16, tag="wt", name="t_wt")
                nc.sync.dma_start(
                    out=wtl[:, 0:kc_n, :],
                    in_=bass.AP(tensor=wtT, offset=q * WKC * 128 * OSL,
                                ap=[[OSL, 128], [128 * OSL, kc_n], [1, OSL]]),
                )
                for kc in range(kc_n):
                    k = q * WKC + kc
                    lhsT = bass.AP(tensor=fap.tensor, offset=fap.offset + k,
                                   ap=[fp0, [1, 4], [128, 32]])
                    for (pt, o0, on) in pybs:
                        nc.tensor.matmul(
                            out=pt[:], lhsT=lhsT,
                            rhs=wtl[:, kc, o0:o0 + on],
                            start=(k == 0), stop=(k == NP - 1),
                            skip_group_check=True)

            yout = work.tile([32, OSL], f32, tag="yout", name="t_yout")
            nc.scalar.copy(out=yout[:, 0:512], in_=pyb0[0:32, :])
            nc.scalar.copy(out=yout[:, 512:OSL], in_=pyb1[0:32, :])
            nc.scalar.dma_start(out=y[:], in_=yout[:])

    _legalize_waits(nc)
    return nc


def _legalize_waits(nc):
    """walrus on this toolchain accepts only one sync wait per non-sequencer
    instruction. Move extra waits onto standalone InstEventSemaphore
    instructions (sequencer-level waits, multi-wait legal) placed just
    before the owning instruction on the same engine."""
    n_moved = 0
    for bb in nc.main_func.blocks:
        out = []
        for inst in bb.instructions:
            si = inst.sync_info
            tn = type(inst).__name__
            if (si is not None and len(si.on_wait) > 1
                    and tn not in ("InstEventSemaphore", "InstNoOp")):
                waits = list(si.on_wait)
                for w in waits[:-1]:
                    ev = mybir.InstNoOp(
                        name=f"lw_{inst.name}_{n_moved}", ins=[], outs=[],
                        engine=inst.engine)
                    ev.sync_info = mybir.SyncInfo(on_wait=[w], on_update=[])
                    nc.register_instruction(ev)
                    out.append(ev)
                    n_moved += 1
                inst.sync_info = mybir.SyncInfo(
                    on_wait=[waits[-1]], on_update=list(si.on_update))
            out.append(inst)
        bb.instructions = out


def _sincos_pe(n, d):
    pos = np.arange(n, dtype=np.float32)[:, None]
    sin_cols, cos_cols = (d + 1) // 2, d // 2
    denom = d / 2.0
    sin_div = np.exp(
        (-math.log(10000.0) * np.arange(sin_cols, dtype=np.float32) / denom)
    ).astype(np.float32)
    cos_div = np.exp(
        (-math.log(10000.0) * np.arange(cos_cols, dtype=np.float32) / denom)
    ).astype(np.float32)
    pe = np.zeros((n, d), dtype=np.float32)
    pe[:, 0::2] = np.sin(pos * sin_div[None, :])
    pe[:, 1::2] = np.cos(pos * cos_div[None, :])
    return pe


def _to_bf16(a):
    import ml_dtypes
    return np.asarray(a, np.float32).astype(ml_dtypes.bfloat16)


def make_in_maps(x, pe_W, pe_b, norm_w, ipa_W, ipb_W, conv_W, conv_b,
                 alpha, beta, gamma, delta, op_W, normf_w, out_W, out_b):
    f = np.float32
    x = np.asarray(x, f)
    x_pad = np.zeros((B, C, LPAD + 4), f)
    x_pad[:, :, :L] = x
    # device layout: [p2*64+c, b_loc, l] with p2=1 rows shifted by 8 along l;
    # 129 windows of 4 (the last is zero padding for the shifted matmuls)
    xcT = x_pad.transpose(1, 0, 2)                     # [c, b, lpad]
    x_dev = np.empty((2, C, B, NW * 4), f)
    x_dev[0] = xcT[:, :, 0:NW * 4]
    x_dev[1] = xcT[:, :, 8:8 + NW * 4]
    x_dev = x_dev.reshape(128, B, NW * 4)

    pw = np.asarray(pe_W, f).reshape(D, C, P)          # [d, c, p]
    t = pw.transpose(1, 2, 0)                          # [c, p, d]
    pe8 = np.ascontiguousarray(
        t.reshape(C, 2, 8, D).transpose(2, 1, 0, 3).reshape(8, 128, 128))
    pe8 = np.ascontiguousarray(pe8.transpose(1, 0, 2))  # [pp, j, d]

    posb = _sincos_pe(NP, D).T + np.asarray(pe_b, f)[:, None]   # [128, 125]
    posbB = np.zeros((128, BLOC, 128), f)
    posbB[:, :, :NP] = posb[:, None, :]
    posbB = posbB.reshape(128, TOK)

    ident = np.eye(128, dtype=f)
    sqd = np.full((128, 128), SQRTD, f)
    misc = np.zeros((128, 2), f)
    misc[:, 0] = 1.0
    misc[:, 1] = np.asarray(normf_w, f)

    nw = np.asarray(norm_w, f)                          # [NL, D]
    ipa = np.asarray(ipa_W, f)                          # [NL, INNER, D]
    cw = np.asarray(conv_W, f)[:, :, 0, :]              # [NL, INNER, K5]
    w5 = (ipa.transpose(0, 2, 1)[:, None, :, :]         # [NL, 1, D, INNER]
          * cw.transpose(0, 2, 1)[:, :, None, :]        # [NL, K5, 1, INNER]
          * nw[:, None, :, None])                       # [NL, K5, D, INNER]
    bwh = np.asarray(ipb_W, f).transpose(0, 2, 1) * nw[:, :, None]  # [NL, D, INNER]
    owh = np.asarray(op_W, f).transpose(0, 2, 1).reshape(NL, 2, 128, D)

    asig = 1.0 / (1.0 + np.exp(-np.asarray(alpha, np.float64)))
    gb = np.asarray(gamma, np.float64) * np.asarray(beta, np.float64)
    p4 = np.stack([np.asarray(conv_b, f),
                   asig.astype(f),
                   gb.astype(f),
                   np.asarray(delta, f)], axis=0)       # [4, NL, INNER]

    layer_cols = []
    for l in range(NL):
        w5l = w5[l].transpose(1, 0, 2).reshape(128, K5 * INNER)   # [d,(k i)]
        bwl = bwh[l].reshape(128, INNER)                          # [d, i]
        # residual lives in PSUM as h' = h * 2^-l: fold 2^-(l+1) into op_W
        owl = np.ascontiguousarray(
            owh[l].transpose(2, 0, 1)).reshape(128, 2 * D) * (0.5 ** (l + 1))
        scl = np.ascontiguousarray(
            p4[:, l].reshape(4, 2, 128).transpose(2, 0, 1)).reshape(128, 8)
        layer_cols.append(np.concatenate(
            [w5l, bwl, owl.astype(f), scl], axis=1))

    oW = np.asarray(out_W, f)
    pe8h = _to_bf16(pe8.reshape(128, 8 * 128))
    wpk = np.ascontiguousarray(np.concatenate(
        [posbB, ident, sqd, misc] + layer_cols, axis=1))
    assert wpk.shape[1] == WCOLS, wpk.shape
    in_maps = []
    for r in range(NCORES):
        wphk = np.concatenate([
            _to_bf16(x_dev[:, r * BLOC:(r + 1) * BLOC, :].reshape(
                128, BLOC * NW * 4)),
            pe8h,
        ], axis=1)
        assert wphk.shape[1] == HCOLS, wphk.shape
        in_maps.append({
            "wph": np.ascontiguousarray(wphk),
            "wpack": wpk,
            "wt": _to_bf16(oW[r * OSL:(r + 1) * OSL].T),
        })
    return in_maps


def kernel(**inputs):
    global _PROG
    if _PROG is None:
        _PROG = build_program()
    in_maps = make_in_maps(**inputs)
    res = run_bass_kernel_spmd(_PROG, in_maps, list(range(NCORES)))
    return assemble_output([res.results[r]["y"] for r in range(NCORES)],
                           inputs["out_b"])


def assemble_output(ys, out_b=None):
    y = np.concatenate([np.asarray(yr).reshape(B, OSL) for yr in ys], axis=1)
    if out_b is not None:
        y = y + np.asarray(out_b, np.float32)[None, :]
    return y.reshape(B, C, F)


# revision 19
# speedup vs baseline: 1.2700x; 1.0093x over previous
"""Trainium2 Bass kernel for nn_CMambaSlim.

Strategy (8 NeuronCores):
  - Data-parallel trunk: each core runs the CMamba trunk (patch embed, 4
    mamba blocks, final RMSNorm) on B/8 = 4 batch samples, fp32/f32r.
  - AllGather of the flattened features (1 MB bf16) across the 8 cores.
  - Tensor-parallel output layer: core r streams rows [r*768, (r+1)*768) of
    out_W in bf16 (pre-transposed on host to [16000, 768]) and computes
    y[:, r*768:(r+1)*768]. out_b is added on the host during unsharding.

Schedule notes (CoreSim cost model):
  - All DMAs serialize on the DMA-engine device and hold the issuing
    engine's sequencer. SP's queue carries ONLY the wpack loads + the 32
    streamed weight chunks, so weight prefetch runs through the trunk and
    the AllGather. ccin/flatT/y DMAs issue from the Activation engine.
  - The residual stream h lives in PSUM: the out-projection matmuls
    accumulate straight into it (h' = h * 2^-l with the 2^-(l+1) folded
    into op_W host-side; rmsnorm is scale-invariant so only the eps
    constant needs a per-layer 4^-l).
  - The trunk is split into two independent 256-token halves (the conv
    windows are separated by a zeroed 4-column gap in hn), pipelined
    through ACT (square/sigmoid/aM), DVE (rstd/hn/u/gg/dab), Pool
    (scan/g0, SBUF-only operands), and PE.
  - ACT stays on the sigmoid table the whole trunk; rstd is computed on
    DVE as (ssum + D*eps_l)^-0.5 via AluOp.pow.
"""

import math
import os
import sys

import numpy as np

for _p in ("/opt/trn_rl_repo", "/root/.axon_site/_ro/trn_rl_repo"):
    if os.path.isdir(_p) and _p not in sys.path:
        sys.path.insert(0, _p)
        break

import concourse.bass as bass
import concourse.tile as tile
from concourse import mybir
from concourse.bass_utils import run_bass_kernel_spmd

# Model dims (hardcoded per problem spec)
B, C, L = 32, 64, 512
P, S = 16, 4
NP = 125
D = 128
INNER = 256
K5 = 5
NL = 4
F = 96
EPS = 1e-5

NCORES = 8
BLOC = B // NCORES            # 4 samples per core
OSL = (C * F) // NCORES       # 768 output cols per core
TOK = BLOC * 128              # padded token span (125 valid + 3 pad per sample)
HTOK = TOK // 2               # 256 tokens per pipelined half
HNW = 2 + HTOK + 4 + HTOK + 2  # hn with guards + inter-half gap = 520
LPAD = 520                    # x padded along L so the +8-shifted copy stays in bounds
NF = NP * D                   # 16000 contraction size
WKC = 4                       # k-chunks per weight-stream DMA
NQ = (NP + WKC - 1) // WKC    # 32 chunks (31 full + 1 partial)
WRING = 18                    # weight ring depth (chunks prefetchable)

f32 = mybir.dt.float32
f32r = mybir.dt.float32r
bf16 = mybir.dt.bfloat16
AF = mybir.ActivationFunctionType
OP = mybir.AluOpType

_PROG = None

SQRTD = math.sqrt(float(D))

# wph: bf16 embed inputs (x windows + patch-embed W), loaded first
NW = 129                                   # x windows (last is zero padding)
HOFF_PE8 = BLOC * NW * 4                   # 2064 cols of x
HCOLS = HOFF_PE8 + 8 * 128                 # + 1024 cols of patch-embed W
# wpack: fp32 consts (embed tail + one DMA per layer)
WOFF_IDN = TOK                             # 512 cols: posb broadcast to (b,k)
WOFF_SQD = WOFF_IDN + 128                  # 128 cols: identity matrix
WOFF_MISC = WOFF_SQD + 128                 # 128 cols: sqrt(D) everywhere
WOFF_L0 = WOFF_MISC + 2                    # col 0: 1.0 (onesD), col 1: normf_w
LCOLS = K5 * INNER + INNER + 2 * D + 8     # 1800 cols per layer
WCOLS = WOFF_L0 + NL * LCOLS               # 7970


def build_program():
    nc = bass.Bass(num_devices=NCORES)

    wph = nc.declare_dram_parameter("wph", [128, HCOLS], bf16, isOutput=False)
    wpack = nc.declare_dram_parameter("wpack", [128, WCOLS], f32, isOutput=False)
    wt = nc.declare_dram_parameter("wt", [NF, OSL], bf16, isOutput=False)
    y = nc.declare_dram_parameter("y", [B, OSL], f32, isOutput=True)

    wtT = wt[:].tensor

    with tile.TileContext(nc) as tc:
        with (
            tc.tile_pool(name="const", bufs=1) as const,
            tc.tile_pool(name="work", bufs=1) as work,
            tc.tile_pool(name="wring", bufs=WRING) as wring,
            tc.tile_pool(name="ps", bufs=1, space="PSUM") as ps,
            tc.tile_pool(name="dram", bufs=1, space="DRAM") as dram,
        ):
            # ---------------- constant loads (embed parts, then per layer) ---
            # posbB/ident first (gates the first embed matmul), then x/pe8
            wp = const.tile([128, WCOLS], f32r)
            nc.sync.dma_start(out=wp[:, 0:WOFF_L0],
                              in_=wpack[:, 0:WOFF_L0].bitcast(f32r))
            wh = const.tile([128, HCOLS], bf16)
            nc.sync.dma_start(out=wh[:], in_=wph[:])
            for l in range(NL):
                c0 = WOFF_L0 + l * LCOLS
                nc.sync.dma_start(out=wp[:, c0:c0 + LCOLS],
                                  in_=wpack[:, c0:c0 + LCOLS].bitcast(f32r))

            xO4 = wh[:, 0:HOFF_PE8].rearrange(
                "p (b k s) -> p b k s", b=BLOC, s=4)          # [128, 4, 129, 4]
            pe8sb = wh[:, HOFF_PE8:HCOLS].rearrange("p (j d) -> p j d", j=8)
            posbB = wp[:, 0:WOFF_IDN]                          # [128, 512] (b,k)
            ident = wp[:, WOFF_IDN:WOFF_SQD]                   # I_128
            sqrtDrow = wp[0:1, WOFF_SQD:WOFF_SQD + 128]        # value sqrt(D)
            onesD = wp[:, WOFF_MISC:WOFF_MISC + 1]             # value 1.0
            normf = wp[:, WOFF_MISC + 1:WOFF_MISC + 2].bitcast(f32)

            def lview(l):
                b0 = WOFF_L0 + l * LCOLS
                w5 = wp[:, b0:b0 + K5 * INNER].rearrange(
                    "p (k i) -> p k i", k=K5)
                bw = wp[:, b0 + K5 * INNER:b0 + K5 * INNER + INNER]
                ow = wp[:, b0 + K5 * INNER + INNER:
                        b0 + K5 * INNER + INNER + 2 * D].rearrange(
                    "p (c d) -> p c d", c=2)
                scal = wp[:, b0 + LCOLS - 8:b0 + LCOLS].bitcast(f32).rearrange(
                    "p (s c) -> p s c", s=4)
                return w5, bw, ow, scal

            # mask01: 1 everywhere, 0 at each sample's k=0 column (scan reset)
            mask01 = const.tile([128, TOK], f32)
            nc.vector.memset(mask01[:], 1.0)
            for bq in range(BLOC):
                nc.vector.memset(mask01[:, bq * 128:bq * 128 + 1], 0.0)

            # residual stream h' lives in PSUM; out-projections accumulate
            # into it (never stopped). h' = h * 2^-l, exact via scaled op_W.
            hps = ps.tile([128, TOK], f32, tag="h", name="t_h")
            hps_bk = hps[:].rearrange("p (b k) -> p b k", b=BLOC)

            # normalized-input tile: [2 guard | half0 | 4 gap | half1 | 2 guard]
            hn = const.tile([128, HNW], f32)
            nc.vector.memset(hn[:, 0:2], 0.0)
            nc.vector.memset(hn[:, 2 + HTOK:2 + HTOK + 4], 0.0)
            nc.vector.memset(hn[:, HNW - 2:], 0.0)
            hnr = hn[:].bitcast(f32r)
            HNS = (2, 2 + HTOK + 4)        # hn write offset per half
            HR = (0, HTOK)                 # token-range start per half

            # ---------------- patch embedding (into h PSUM) ----------------
            # identity @ posbB first: start=True pending-zeros the whole
            # region and this matmul touches every byte.
            nc.tensor.matmul(out=hps[:], lhsT=ident, rhs=posbB,
                             start=True, stop=False, skip_group_check=True)
            for j in range(8):
                jq, jr = j // 4, j % 4
                rhs = xO4[:, :, jq:jq + 128, jr]
                nc.tensor.matmul(
                    out=hps[:], lhsT=pe8sb[:, j, :],
                    rhs=rhs, start=False, stop=False, skip_group_check=True)
            # zero the 3 pad tokens per sample (windows 125..127 hold junk)
            nc.vector.memset(hps_bk[:, :, 125:128], 0.0)

            # ---------------- mamba layers (two pipelined halves) -----------
            def emit_layer(l):
                w5sb, bwsb, owsb, scalsb = lview(l)
                epsl = float(D) * EPS * (0.25 ** l)
                sq = work.tile([128, TOK], f32, tag="sq", name=f"sq_{l}")
                rstd = work.tile([1, TOK], f32, tag="rstd", name=f"rstd_{l}")
                pssum = ps.tile([1, TOK], f32, tag="pss", bufs=2, name=f"pss_{l}")
                prstd = ps.tile([128, TOK], f32, tag="prstd", name=f"prstd_{l}")
                pa = [ps.tile([128, TOK], f32, tag=f"pa{ic}", name=f"pa{ic}_{l}")
                      for ic in range(2)]
                pb = [ps.tile([128, TOK], f32, tag=f"pb{ic}", name=f"pb{ic}_{l}")
                      for ic in range(2)]
                aM, sg, ab, sc, dab, gg = ({}, {}, {}, {}, {}, {})
                for ic in range(2):
                    aM[ic] = work.tile([128, TOK], f32, tag=f"am{ic}", bufs=2,
                                       name=f"am{ic}_{l}")
                    sg[ic] = work.tile([128, TOK], f32, tag=f"sg{ic}", name=f"sg{ic}_{l}")
                    ab[ic] = work.tile([128, TOK], f32, tag=f"ab{ic}", name=f"ab{ic}_{l}")
                    sc[ic] = work.tile([128, TOK], f32, tag=f"s{ic}", name=f"s{ic}_{l}")
                    dab[ic] = work.tile([128, TOK], f32, tag=f"dab{ic}", name=f"dab{ic}_{l}")
                    gg[ic] = work.tile([128, TOK], f32, tag=f"g{ic}", name=f"g{ic}_{l}")

                # aM first: depends only on constants, fills ACT while the
                # previous layer's gate phase runs (bufs=2 on the am tags)
                for hh in range(2):
                    r0 = HR[hh]
                    for ic in range(2):
                        nc.scalar.activation(
                            out=aM[ic][:, r0:r0 + HTOK], in_=mask01[:, r0:r0 + HTOK],
                            func=AF.Copy, scale=scalsb[:, 1, ic:ic + 1])
                for hh in range(2):
                    r0 = HR[hh]
                    nc.scalar.activation(out=sq[:, r0:r0 + HTOK],
                                         in_=hps[:, r0:r0 + HTOK], func=AF.Square)
                for hh in range(2):
                    r0 = HR[hh]
                    nc.tensor.matmul(
                        out=pssum[0:1, r0:r0 + HTOK], lhsT=onesD,
                        rhs=sq[:, r0:r0 + HTOK].bitcast(f32r),
                        start=True, stop=True, skip_group_check=True)
                for hh in range(2):
                    r0 = HR[hh]
                    nc.vector.tensor_scalar(
                        out=rstd[0:1, r0:r0 + HTOK], in0=pssum[0:1, r0:r0 + HTOK],
                        scalar1=epsl, scalar2=-0.5, op0=OP.add, op1=OP.pow)
                for hh in range(2):
                    r0 = HR[hh]
                    nc.tensor.matmul(
                        out=prstd[:, r0:r0 + HTOK], lhsT=sqrtDrow,
                        rhs=rstd[0:1, r0:r0 + HTOK].bitcast(f32r),
                        start=True, stop=True, skip_group_check=True)
                for hh in range(2):
                    r0, h0 = HR[hh], HNS[hh]
                    nc.vector.tensor_tensor(
                        out=hn[:, h0:h0 + HTOK], in0=hps[:, r0:r0 + HTOK],
                        in1=prstd[:, r0:r0 + HTOK], op=OP.mult)
                for hh in range(2):
                    h0 = HNS[hh]
                    for ic in range(2):
                        for dk in range(K5):
                            nc.tensor.matmul(
                                out=pa[ic][:, HR[hh]:HR[hh] + HTOK],
                                lhsT=w5sb[:, dk, ic * 128:(ic + 1) * 128],
                                rhs=hnr[:, h0 - 2 + dk:h0 - 2 + dk + HTOK],
                                start=(dk == 0), stop=(dk == K5 - 1),
                                skip_group_check=True)
                    for ic in range(2):
                        nc.tensor.matmul(
                            out=pb[ic][:, HR[hh]:HR[hh] + HTOK],
                            lhsT=bwsb[:, ic * 128:(ic + 1) * 128],
                            rhs=hnr[:, h0:h0 + HTOK],
                            start=True, stop=True, skip_group_check=True)
                # gate phase: half-major so the two halves pipeline cleanly
                for hh in range(2):
                    r0 = HR[hh]
                    for ic in range(2):
                        # silu(z) = z * sigmoid(z), z = conv + conv_b
                        nc.scalar.activation(
                            out=sg[ic][:, r0:r0 + HTOK], in_=pa[ic][:, r0:r0 + HTOK],
                            func=AF.Sigmoid, bias=scalsb[:, 0, ic:ic + 1], scale=1.0)
                    for ic in range(2):
                        nc.vector.scalar_tensor_tensor(
                            out=ab[ic][:, r0:r0 + HTOK], in0=pa[ic][:, r0:r0 + HTOK],
                            scalar=scalsb[:, 0, ic:ic + 1],
                            in1=sg[ic][:, r0:r0 + HTOK], op0=OP.add, op1=OP.mult)
                    for ic in range(2):
                        # scan: state = aM*state + u (Pool; SBUF operands only)
                        nc.gpsimd.tensor_tensor_scan(
                            out=sc[ic][:, r0:r0 + HTOK], data0=aM[ic][:, r0:r0 + HTOK],
                            data1=ab[ic][:, r0:r0 + HTOK], initial=0.0,
                            op0=OP.mult, op1=OP.add)
                    for ic in range(2):
                        nc.vector.tensor_scalar_mul(
                            out=dab[ic][:, r0:r0 + HTOK], in0=ab[ic][:, r0:r0 + HTOK],
                            scalar1=scalsb[:, 3, ic:ic + 1])
                    for ic in range(2):
                        # g0 = gamma*beta*s + dab (Pool; SBUF operands only)
                        nc.gpsimd.scalar_tensor_tensor(
                            out=sc[ic][:, r0:r0 + HTOK], in0=sc[ic][:, r0:r0 + HTOK],
                            scalar=scalsb[:, 2, ic:ic + 1],
                            in1=dab[ic][:, r0:r0 + HTOK], op0=OP.mult, op1=OP.add)
                    for ic in range(2):
                        # pads stay zero: hn pads are zero so pb pads are zero
                        nc.vector.tensor_tensor(
                            out=gg[ic][:, r0:r0 + HTOK], in0=sc[ic][:, r0:r0 + HTOK],
                            in1=pb[ic][:, r0:r0 + HTOK], op=OP.mult)
                    for ic in range(2):
                        # residual: h' += 2^-(l+1) * oW @ g (scale folded into oW)
                        nc.tensor.matmul(
                            out=hps[:, r0:r0 + HTOK], lhsT=owsb[:, ic, :],
                            rhs=gg[ic][:, r0:r0 + HTOK].bitcast(f32r),
                            start=False, stop=False, skip_group_check=True)

            for l in range(NL):
                emit_layer(l)

            # ---------------- final rmsnorm ----------------
            epsf = float(D) * EPS * (0.25 ** NL)
            sqf = work.tile([128, TOK], f32, tag="sq", name="t_sqf")
            rstdf = work.tile([1, TOK], f32, tag="rstd", name="t_rstdf")
            pssumf = ps.tile([1, TOK], f32, tag="pss", bufs=2, name="t_pssf")
            prstdf = ps.tile([128, TOK], f32, tag="prstd", name="t_prstdf")
            hf = work.tile([128, TOK], bf16, tag="hf", name="t_hf")
            for hh in range(2):
                r0 = HR[hh]
                nc.scalar.activation(out=sqf[:, r0:r0 + HTOK],
                                     in_=hps[:, r0:r0 + HTOK], func=AF.Square)
            for hh in range(2):
                r0 = HR[hh]
                nc.tensor.matmul(
                    out=pssumf[0:1, r0:r0 + HTOK], lhsT=onesD,
                    rhs=sqf[:, r0:r0 + HTOK].bitcast(f32r),
                    start=True, stop=True, skip_group_check=True)
            for hh in range(2):
                r0 = HR[hh]
                nc.vector.tensor_scalar(
                    out=rstdf[0:1, r0:r0 + HTOK], in0=pssumf[0:1, r0:r0 + HTOK],
                    scalar1=epsf, scalar2=-0.5, op0=OP.add, op1=OP.pow)
            for hh in range(2):
                r0 = HR[hh]
                nc.tensor.matmul(
                    out=prstdf[:, r0:r0 + HTOK], lhsT=sqrtDrow,
                    rhs=rstdf[0:1, r0:r0 + HTOK].bitcast(f32r),
                    start=True, stop=True, skip_group_check=True)
            for hh in range(2):
                r0 = HR[hh]
                nc.vector.scalar_tensor_tensor(
                    out=hf[:, r0:r0 + HTOK], in0=hps[:, r0:r0 + HTOK],
                    scalar=normf, in1=prstdf[:, r0:r0 + HTOK],
                    op0=OP.mult, op1=OP.mult)

            # ---------------- all-gather the features (bf16) ----------------
            ccin = dram.tile([128, TOK], bf16)
            nc.scalar.dma_start(out=ccin[:], in_=hf[:])
            # inner dim padded so the gathered blocks stay stride-separated
            TOKP = TOK + 8
            ccout = dram.tile([NCORES, 128, TOKP], bf16, addr_space="Shared")
            nc.gpsimd.collective_compute(
                "AllGather", OP.bypass,
                replica_groups=[list(range(NCORES))],
                ins=[ccin[:].opt()], outs=[ccout[:, :, 0:TOK]])
            # flatT[d, b, k] (k padded to 128; pads are zero), b = r*BLOC + b4
            flatT = const.tile([128, B, 128], bf16)
            nc.scalar.dma_start(
                out=flatT[:].rearrange("p (r x) k -> p r (x k)", r=NCORES),
                in_=bass.AP(tensor=ccout[:].tensor, offset=ccout[:].offset,
                            ap=[[TOKP, 128], [128 * TOKP, NCORES], [1, TOK]]),
            )
            fap = flatT[:]
            fp0 = list(fap.ap[0])

            # ---------------- streamed output matmul ----------------
            # y[b, o] accumulated over the 125 (k, d) chunks. Stationary
            # operand = flatT columns (j, b) at offset k: output row j*32+b
            # holds sum_d flat[d, b, k+j] * wt_k[d, o]; rows 0..31 / j=0 are
            # the real batch rows, the rest M-padding. Moving operand = the
            # streamed bf16 W tile.
            pyb0 = ps.tile([128, 512], f32, tag="pa0", name="t_pyb0")
            pyb1 = ps.tile([128, OSL - 512], f32, tag="pa1", name="t_pyb1")
            pybs = ((pyb0, 0, 512), (pyb1, 512, OSL - 512))
            for q in range(NQ):
                kc_n = min(WKC, NP - q * WKC)
                wtl = wring.tile([128, WKC, OSL], bf16, tag="wt", name="t_wt")
                nc.sync.dma_start(
                    out=wtl[:, 0:kc_n, :],
                    in_=bass.AP(tensor=wtT, offset=q * WKC * 128 * OSL,
                                ap=[[OSL, 128], [128 * OSL, kc_n], [1, OSL]]),
                )
                for kc in range(kc_n):
                    k = q * WKC + kc
                    lhsT = bass.AP(tensor=fap.tensor, offset=fap.offset + k,
                                   ap=[fp0, [1, 4], [128, 32]])
                    for (pt, o0, on) in pybs:
                        nc.tensor.matmul(
                            out=pt[:], lhsT=lhsT,
                            rhs=wtl[:, kc, o0:o0 + on],
                            start=(k == 0), stop=(k == NP - 1),
                            skip_group_check=True)

            yout = work.tile([32, OSL], f32, tag="yout", name="t_yout")
            nc.scalar.copy(out=yout[:, 0:512], in_=pyb0[0:32, :])
            nc.scalar.copy(out=yout[:, 512:OSL], in_=pyb1[0:32, :])
            nc.scalar.dma_start(out=y[:], in_=yout[:])

    _legalize_waits(nc)
    return nc


def _legalize_waits(nc):
    """walrus on this toolchain accepts only one sync wait per non-sequencer
    instruction. Move extra waits onto standalone InstEventSemaphore
    instructions (sequencer-level waits, multi-wait legal) placed just
    before the owning instruction on the same engine."""
    n_moved = 0
    for bb in nc.main_func.blocks:
        out = []
        for inst in bb.instructions:
            si = inst.sync_info
            tn = type(inst).__name__
            if (si is not None and len(si.on_wait) > 1
                    and tn not in ("InstEventSemaphore", "InstNoOp")):
                waits = list(si.on_wait)
                for w in waits[:-1]:
                    ev = mybir.InstNoOp(
                        name=f"lw_{inst.name}_{n_moved}", ins=[], outs=[],
                        engine=inst.engine)
                    ev.sync_info = mybir.SyncInfo(on_wait=[w], on_update=[])
                    nc.register_instruction(ev)
                    out.append(ev)
                    n_moved += 1
                inst.sync_info = mybir.SyncInfo(
                    on_wait=[waits[-1]], on_update=list(si.on_update))
            out.append(inst)
        bb.instructions = out


def _sincos_pe(n, d):
    pos = np.arange(n, dtype=np.float32)[:, None]
    sin_cols, cos_cols = (d + 1) // 2, d // 2
    denom = d / 2.0
    sin_div = np.exp(
        (-math.log(10000.0) * np.arange(sin_cols, dtype=np.float32) / denom)
    ).astype(np.float32)
    cos_div = np.exp(
        (-math.log(10000.0) * np.arange(cos_cols, dtype=np.float32) / denom)
    ).astype(np.float32)
    pe = np.zeros((n, d), dtype=np.float32)
    pe[:, 0::2] = np.sin(pos * sin_div[None, :])
    pe[:, 1::2] = np.cos(pos * cos_div[None, :])
    return pe


def _to_bf16(a):
    import ml_dtypes
    return np.asarray(a, np.float32).astype(ml_dtypes.bfloat16)


def make_in_maps(x, pe_W, pe_b, norm_w, ipa_W, ipb_W, conv_W, conv_b,
                 alpha, beta, gamma, delta, op_W, normf_w, out_W, out_b):
    f = np.float32
    x = np.asarray(x, f)
    x_pad = np.zeros((B, C, LPAD + 4), f)
    x_pad[:, :, :L] = x
    # device layout: [p2*64+c, b_loc, l] with p2=1 rows shifted by 8 along l;
    # 129 windows of 4 (the last is zero padding for the shifted matmuls)
    xcT = x_pad.transpose(1, 0, 2)                     # [c, b, lpad]
    x_dev = np.empty((2, C, B, NW * 4), f)
    x_dev[0] = xcT[:, :, 0:NW * 4]
    x_dev[1] = xcT[:, :, 8:8 + NW * 4]
    x_dev = x_dev.reshape(128, B, NW * 4)

    pw = np.asarray(pe_W, f).reshape(D, C, P)          # [d, c, p]
    t = pw.transpose(1, 2, 0)                          # [c, p, d]
    pe8 = np.ascontiguousarray(
        t.reshape(C, 2, 8, D).transpose(2, 1, 0, 3).reshape(8, 128, 128))
    pe8 = np.ascontiguousarray(pe8.transpose(1, 0, 2))  # [pp, j, d]

    posb = _sincos_pe(NP, D).T + np.asarray(pe_b, f)[:, None]   # [128, 125]
    posbB = np.zeros((128, BLOC, 128), f)
    posbB[:, :, :NP] = posb[:, None, :]
    posbB = posbB.reshape(128, TOK)

    ident = np.eye(128, dtype=f)
    sqd = np.full((128, 128), SQRTD, f)
    misc = np.zeros((128, 2), f)
    misc[:, 0] = 1.0
    misc[:, 1] = np.asarray(normf_w, f)

    nw = np.asarray(norm_w, f)                          # [NL, D]
    ipa = np.asarray(ipa_W, f)                          # [NL, INNER, D]
    cw = np.asarray(conv_W, f)[:, :, 0, :]              # [NL, INNER, K5]
    w5 = (ipa.transpose(0, 2, 1)[:, None, :, :]         # [NL, 1, D, INNER]
          * cw.transpose(0, 2, 1)[:, :, None, :]        # [NL, K5, 1, INNER]
          * nw[:, None, :, None])                       # [NL, K5, D, INNER]
    bwh = np.asarray(ipb_W, f).transpose(0, 2, 1) * nw[:, :, None]  # [NL, D, INNER]
    owh = np.asarray(op_W, f).transpose(0, 2, 1).reshape(NL, 2, 128, D)

    asig = 1.0 / (1.0 + np.exp(-np.asarray(alpha, np.float64)))
    gb = np.asarray(gamma, np.float64) * np.asarray(beta, np.float64)
    p4 = np.stack([np.asarray(conv_b, f),
                   asig.astype(f),
                   gb.astype(f),
                   np.asarray(delta, f)], axis=0)       # [4, NL, INNER]

    layer_cols = []
    for l in range(NL):
        w5l = w5[l].transpose(1, 0, 2).reshape(128, K5 * INNER)   # [d,(k i)]
        bwl = bwh[l].reshape(128, INNER)                          # [d, i]
        # residual lives in PSUM as h' = h * 2^-l: fold 2^-(l+1) into op_W
        owl = np.ascontiguousarray(
            owh[l].transpose(2, 0, 1)).reshape(128, 2 * D) * (0.5 ** (l + 1))
        scl = np.ascontiguousarray(
            p4[:, l].reshape(4, 2, 128).transpose(2, 0, 1)).reshape(128, 8)
        layer_cols.append(np.concatenate(
            [w5l, bwl, owl.astype(f), scl], axis=1))

    oW = np.asarray(out_W, f)
    pe8h = _to_bf16(pe8.reshape(128, 8 * 128))
    wpk = np.ascontiguousarray(np.concatenate(
        [posbB, ident, sqd, misc] + layer_cols, axis=1))
    assert wpk.shape[1] == WCOLS, wpk.shape
    in_maps = []
    for r in range(NCORES):
        wphk = np.concatenate([
            _to_bf16(x_dev[:, r * BLOC:(r + 1) * BLOC, :].reshape(
                128, BLOC * NW * 4)),
            pe8h,
        ], axis=1)
        assert wphk.shape[1] == HCOLS, wphk.shape
        in_maps.append({
            "wph": np.ascontiguousarray(wphk),
            "wpack": wpk,
            "wt": _to_bf16(oW[r * OSL:(r + 1) * OSL].T),
        })
    return in_maps


def kernel(**inputs):
    global _PROG
    if _PROG is None:
        _PROG = build_program()
    in_maps = make_in_maps(**inputs)
    res = run_bass_kernel_spmd(_PROG, in_maps, list(range(NCORES)))
    return assemble_output([res.results[r]["y"] for r in range(NCORES)],
                           inputs["out_b"])


def assemble_output(ys, out_b=None):
    y = np.concatenate([np.asarray(yr).reshape(B, OSL) for yr in ys], axis=1)
    if out_b is not None:
        y = y + np.asarray(out_b, np.float32)[None, :]
    return y.reshape(B, C, F)
